# revision 51
# baseline (speedup 1.0000x reference)
"""MLA (multi-head latent attention) Trainium2 kernel, SPMD over 8 NeuronCores.

Sharding: core c = 4*b + g handles batch b and head group g (4 heads),
ALL 2048 query rows.  Causality: query chunk c (512 rows) only attends
key chunks 0..4c+3 (lower triangle), so every core does the same
triangular work -- perfectly balanced, no masks off the diagonal.
Each core emits a PARTIAL out-projection (contraction over its 4 heads'
128-dims); the host sums the 4 partials per batch (+bias).  No
collectives.

On-chip layouts are transposed ([feature, token]) so every matmul
contracts over the partition dim with no on-chip transposes.
rotate_half is folded into host-permuted weight copies; 1/sqrt(dh) into
the q weights; softmax skips the max-pass (scores bounded) and gets its
row-sum from an all-ones matmul over a DVE tree-sum of the exp tiles.
Diagonal score tiles are masked multiplicatively (0/1 bf16) after exp.
"""

import os
import sys
import types

for _p in ("/opt/trn_rl_repo", "/root/.axon_site/_ro/trn_rl_repo"):
    if os.path.isdir(_p) and _p not in sys.path:
        sys.path.append(_p)

import numpy as np
import ml_dtypes

import concourse.bass as bass
import concourse.bacc as bacc_mod
import concourse.mybir as mybir
from concourse.tile import TileContext
from concourse.vector_clock import ScopedClock
from concourse.bass_utils import run_bass_kernel_spmd

F32 = mybir.dt.float32
BF16 = mybir.dt.bfloat16
BF16NP = ml_dtypes.bfloat16

HID, H, LAT, R, DH, C = 2048, 16, 512, 32, 128, 96
B, S = 2, 2048
GH = 4            # heads per core
NQC = 4           # query chunks of 512
NKC = 16          # key chunks of 128


def _patch_tile_drain():
    """The staged walrus rejects a Drain carrying >1 sync-wait. Move the
    TileContext tail-drain waits onto single-wait SP nops."""

    def _drain_and_barrier(self, tick_clock, wait_clock):
        drain_inst = self.nc.sync.drain()
        wait_clock.add_sem_waits(
            drain_inst.ins, ScopedClock({None: tick_clock.global_clock})
        )
        si = drain_inst.ins.sync_info
        if si is not None and len(si.on_wait) > 1:
            waits = list(si.on_wait)
            drain_inst.ins.sync_info = mybir.SyncInfo(
                on_wait=[], on_update=list(si.on_update)
            )
            for w in waits:
                nop = self.nc.sync.nop(nofuse=True)
                nop.ins.sync_info = mybir.SyncInfo(on_wait=[w], on_update=[])
        self.nc.all_engine_barrier()
        assert self.sems is not None
        popped = self.nc._tile_sem_poison_stack.pop()
        assert popped is self._sem_poison
        self.nc.clear_and_free_semaphores(list(self.sems.allocated().values()))
        self.nc.all_engine_barrier()

    TileContext._drain_and_barrier = _drain_and_barrier


def _install_ntff_hook():
    """antenv.axon_hooks is absent in this image; inject it and register the
    ctypes NTFF hook so trace=True / BASS_TRACE can profile."""
    try:
        import antenv

        if "antenv.axon_hooks" not in sys.modules:
            mod = types.ModuleType("antenv.axon_hooks")
            mod._hook = None

            def set_axon_ntff_profile_hook(h):
                mod._hook = h

            def get_axon_ntff_profile_hook():
                return mod._hook

            mod.set_axon_ntff_profile_hook = set_axon_ntff_profile_hook
            mod.get_axon_ntff_profile_hook = get_axon_ntff_profile_hook
            sys.modules["antenv.axon_hooks"] = mod
            antenv.axon_hooks = mod
        boot_dir = "/root/.axon_site/trn_agent_boot"
        so_path = "/opt/axon/libaxon_pjrt.so"
        if os.path.isdir(boot_dir) and os.path.exists(so_path):
            if boot_dir not in sys.path:
                sys.path.append(boot_dir)
            from trn_boot import _ntff_profile_via_ctypes

            hook = _ntff_profile_via_ctypes(so_path)
            if hook is not None:
                sys.modules["antenv.axon_hooks"].set_axon_ntff_profile_hook(hook)
    except Exception:
        pass


_patch_tile_drain()
_install_ntff_hook()


def _dram(nc, name, shape, dtype=F32, out=False):
    return nc.declare_dram_parameter(name, list(shape), dtype, isOutput=out)


def build_nc():
    nc = bacc_mod.Bacc("TRN2")

    xbT = _dram(nc, "xbT", [HID, S], BF16)            # x[b].T
    wd_kvT = _dram(nc, "wd_kvT", [HID, LAT], BF16)    # Wkv_down.T
    wd_qT = _dram(nc, "wd_qT", [HID, LAT], BF16)      # Wq_down.T
    wkr2T = _dram(nc, "wkr2T", [HID, 2 * R], BF16)    # [Wk_rope; rot].T
    wk_pT = _dram(nc, "wk_pT", [LAT, GH * C], BF16)   # 4-head k_c pack .T
    wv_pT = _dram(nc, "wv_pT", [LAT, GH * DH], BF16)  # 4-head v pack .T
    wqc_pT = _dram(nc, "wqc_pT", [LAT, GH * C], BF16)   # 4-head q_c pack /sqrt
    wqr_pT = _dram(nc, "wqr_pT", [LAT, GH * R], BF16)   # 4-head q_rope /sqrt
    wqrr_pT = _dram(nc, "wqrr_pT", [LAT, GH * R], BF16)  # rotated rope /sqrt
    woT = _dram(nc, "woT", [GH * DH, HID], BF16)      # Wo cols for our heads
    cos4_d = _dram(nc, "cos4", [128, S], BF16)        # cos.T tiled 4x
    sin4_d = _dram(nc, "sin4", [128, S], BF16)
    mask4_d = _dram(nc, "mask4", [128, NQC * 512], BF16)  # 0/1 diag masks
    out_d = _dram(nc, "out", [S, HID], out=True)      # partial (4-head) proj

    xbT_r = xbT[:, :].rearrange("(c p two) t -> c p two t", p=128, two=2)
    wd_kvT_r = wd_kvT[:, :].rearrange("(c p two) l -> c p two l", p=128, two=2)
    wd_qT_r = wd_qT[:, :].rearrange("(c p two) l -> c p two l", p=128, two=2)
    wkr2T_r = wkr2T[:, :].rearrange("(c p two) r -> c p two r", p=128, two=2)
    wk_pT_r = wk_pT[:, :].rearrange("(lc p) d -> lc p d", p=128)
    wv_pT_r = wv_pT[:, :].rearrange("(lc p) d -> lc p d", p=128)
    wqc_pT_r = wqc_pT[:, :].rearrange("(lc p) d -> lc p d", p=128)
    wqr_pT_r = wqr_pT[:, :].rearrange("(lc p) d -> lc p d", p=128)
    wqrr_pT_r = wqrr_pT[:, :].rearrange("(lc p) d -> lc p d", p=128)
    woT_r = woT[:, :].rearrange("(hc p) o -> hc p o", p=128)

    with TileContext(nc) as tc:
        with tc.tile_pool(name="perB", bufs=1) as perB, \
             tc.tile_pool(name="lat", bufs=2) as LATP, \
             tc.tile_pool(name="xs", bufs=1) as XS, \
             tc.tile_pool(name="ets", bufs=6) as ETS, \
             tc.tile_pool(name="acc", bufs=8) as ACC, \
             tc.tile_pool(name="rcp", bufs=2) as RCP, \
             tc.tile_pool(name="tmp", bufs=2) as TMP, \
             tc.tile_pool(name="ot", bufs=3) as OT, \
             tc.tile_pool(name="ps_g", bufs=2, space="PSUM") as PSG, \
             tc.tile_pool(name="ps_m", bufs=2, space="PSUM") as PSM, \
             tc.tile_pool(name="ps_s", bufs=2, space="PSUM") as PSS, \
             tc.tile_pool(name="ps_c", bufs=2, space="PSUM") as PSC:

            # ---------- persistent SBUF ----------
            krT = perB.tile([32, S], BF16, tag="krT", name="krT")
            kT = perB.tile([128, GH, S], BF16, tag="kT", name="kT")
            vG = perB.tile([128, NKC, GH * DH], BF16, tag="vG", name="vG")
            qT = perB.tile([128, GH, S], BF16, tag="qT", name="qT")
            ctxT = perB.tile([128, GH, S], BF16, tag="ctxT", name="ctxT")
            cos4 = perB.tile([128, S], BF16, tag="cos4", name="cos4")
            sin4 = perB.tile([128, S], BF16, tag="sin4", name="sin4")
            mask4 = perB.tile([128, NQC * 512], BF16, tag="mask4", name="mask4")
            onesb = perB.tile([128, 128], BF16, tag="ones", name="ones")
            wk_sb = perB.tile([128, 4, GH * C], BF16, tag="wk", name="wk")
            wv_sb = perB.tile([128, 4, GH * DH], BF16, tag="wv", name="wv")
            wqc_sb = perB.tile([128, 4, GH * C], BF16, tag="wqc", name="wqc")
            wqr_sb = perB.tile([128, 4, GH * R], BF16, tag="wqr", name="wqr")
            wqrr_sb = perB.tile([128, 4, GH * R], BF16, tag="wqrr", name="wqrr")

            # down-proj weights: released after phase A(3), wo loaded after.
            # Per-hc tiles so the first matmuls wait only on their own slice;
            # kv weights first (the very first accumulation pass).
            WD = tc.alloc_tile_pool(name="wd", bufs=1, side="right")
            wdkv = [WD.tile([128, 2, LAT], BF16, tag=f"wdkv{hc}",
                            name=f"wdkv{hc}") for hc in range(8)]
            wdq = [WD.tile([128, 2, LAT], BF16, tag=f"wdq{hc}",
                           name=f"wdq{hc}") for hc in range(8)]
            wkr = [WD.tile([128, 2, 2 * R], BF16, tag=f"wkr{hc}",
                           name=f"wkr{hc}") for hc in range(8)]

            def load_wd():
                for hc in range(8):
                    nc.sync.dma_start(wdkv[hc][:], wd_kvT_r[hc])
                for hc in range(8):
                    nc.sync.dma_start(wkr[hc][:], wkr2T_r[hc])
                    nc.sync.dma_start(wdq[hc][:], wd_qT_r[hc])

            def load_x(tq):
                tsl = slice(tq * 512, (tq + 1) * 512)
                xt = [XS.tile([128, 2, 512], BF16, tag=f"xf{hc}",
                              name=f"xf{hc}") for hc in range(8)]
                for hc in range(8):
                    nc.sync.dma_start(xt[hc][:], xbT_r[hc][:, :, tsl])
                return xt

            WO = [None]  # box for the late wo pool
            wo_sb = [None]

            def load_cossin():
                nc.sync.dma_start(cos4[:], cos4_d[:, :])
                nc.sync.dma_start(sin4[:], sin4_d[:, :])

            def load_aux_weights():
                nc.sync.dma_start(mask4[:], mask4_d[:, :])
                nc.gpsimd.memset(onesb[:], 1.0)
                for lc in range(4):
                    nc.sync.dma_start(wk_sb[:, lc, :], wk_pT_r[lc])
                    nc.sync.dma_start(wv_sb[:, lc, :], wv_pT_r[lc])
                    nc.sync.dma_start(wqc_sb[:, lc, :], wqc_pT_r[lc])
                    nc.sync.dma_start(wqr_sb[:, lc, :], wqr_pT_r[lc])
                    nc.sync.dma_start(wqrr_sb[:, lc, :], wqrr_pT_r[lc])

            # ---------------- phase emitters ----------------
            def phA(tq, xt=None):
                """latents for token quarter tq: kv_lat, roped k_rope, q_lat.
                Returns the per-quarter latent tiles for phB(tq)."""
                tsl = slice(tq * 512, (tq + 1) * 512)
                if xt is None:
                    xt = load_x(tq)
                kv_t = LATP.tile([128, 4, 512], BF16, tag="kvlat",
                                 name="kvlat")
                q_t = LATP.tile([128, 4, 512], BF16, tag="qlat", name="qlat")

                # kv_lat: 4 lc passes, 2 rotating psum banks
                for lc in range(4):
                    ps = PSG.tile([128, 512], F32, tag="g", name=f"pkv{lc}")
                    for hc in range(8):
                        for two in range(2):
                            nc.tensor.matmul(
                                ps[:],
                                lhsT=wdkv[hc][:, two, lc * 128:(lc + 1) * 128],
                                rhs=xt[hc][:, two, :],
                                start=(hc == 0 and two == 0),
                                stop=(hc == 7 and two == 1),
                            )
                    nc.vector.tensor_copy(kv_t[:, lc, :], ps[:])
                # k_rope pass (64 rows: [rope; rot]); combine in place
                pkr = PSG.tile([64, 512], F32, tag="g", name="pkr")
                for hc in range(8):
                    for two in range(2):
                        nc.tensor.matmul(
                            pkr[:],
                            lhsT=wkr[hc][:, two, :],
                            rhs=xt[hc][:, two, :],
                            start=(hc == 0 and two == 0),
                            stop=(hc == 7 and two == 1),
                        )
                nc.vector.tensor_mul(pkr[0:32, :], pkr[0:32, :],
                                     cos4[0:32, tsl])
                tkr = TMP.tile([32, 512], F32, tag="tkr", name="tkr")
                nc.vector.tensor_mul(tkr[:], pkr[32:64, :], sin4[0:32, tsl])
                nc.vector.tensor_add(krT[:, tsl], pkr[0:32, :], tkr[:])
                # q_lat: 4 lc passes
                for lc in range(4):
                    ps = PSG.tile([128, 512], F32, tag="g", name=f"pq{lc}")
                    for hc in range(8):
                        for two in range(2):
                            nc.tensor.matmul(
                                ps[:],
                                lhsT=wdq[hc][:, two, lc * 128:(lc + 1) * 128],
                                rhs=xt[hc][:, two, :],
                                start=(hc == 0 and two == 0),
                                stop=(hc == 7 and two == 1),
                            )
                    nc.vector.tensor_copy(q_t[:, lc, :], ps[:])
                return kv_t, q_t

            def phB_k(tq, kv_t):
                """k_c per head (96 content rows) + shared roped k_rope."""
                tsl = slice(tq * 512, (tq + 1) * 512)
                for h in range(GH):
                    ps = PSM.tile([128, 512], F32, tag="m", name=f"pk{h}")
                    for lc in range(4):
                        nc.tensor.matmul(
                            ps[0:C, :],
                            lhsT=wk_sb[:, lc, h * C:(h + 1) * C],
                            rhs=kv_t[:, lc, :],
                            start=(lc == 0), stop=(lc == 3),
                        )
                    nc.vector.tensor_copy(kT[0:C, h, tsl], ps[0:C, :])
                for h in range(GH):
                    nc.sync.dma_start(kT[C:128, h, tsl], krT[:, tsl])

            def phB_v(tq, kv_t):
                """v: 4 token sub-chunks of 128, out = [t, 4h*128]."""
                for t2 in range(4):
                    kc = tq * 4 + t2
                    ps = PSM.tile([128, 512], F32, tag="m", name=f"pv{t2}")
                    for lc in range(4):
                        nc.tensor.matmul(
                            ps[:],
                            lhsT=kv_t[:, lc, t2 * 128:(t2 + 1) * 128],
                            rhs=wv_sb[:, lc, :],
                            start=(lc == 0), stop=(lc == 3),
                        )
                    nc.vector.tensor_copy(vG[:, kc, :], ps[:])

            def phB_qc(tq, q_t):
                """q_c per head."""
                tsl = slice(tq * 512, (tq + 1) * 512)
                for h in range(GH):
                    ps = PSM.tile([128, 512], F32, tag="m", name=f"pqc{h}")
                    for lc in range(4):
                        nc.tensor.matmul(
                            ps[0:C, :],
                            lhsT=wqc_sb[:, lc, h * C:(h + 1) * C],
                            rhs=q_t[:, lc, :],
                            start=(lc == 0), stop=(lc == 3),
                        )
                    nc.vector.tensor_copy(qT[0:C, h, tsl], ps[0:C, :])

            def phB_qr(tq, q_t):
                """q_rope: stacked 4h x 32 rope + rot; combine, scatter."""
                tsl = slice(tq * 512, (tq + 1) * 512)
                psr = PSM.tile([128, 512], F32, tag="m", name="pqr")
                psrr = PSM.tile([128, 512], F32, tag="m", name="pqrr")
                for lc in range(4):
                    nc.tensor.matmul(
                        psr[:], lhsT=wqr_sb[:, lc, :],
                        rhs=q_t[:, lc, :],
                        start=(lc == 0), stop=(lc == 3),
                    )
                for lc in range(4):
                    nc.tensor.matmul(
                        psrr[:], lhsT=wqrr_sb[:, lc, :],
                        rhs=q_t[:, lc, :],
                        start=(lc == 0), stop=(lc == 3),
                    )
                t2b = TMP.tile([128, 512], F32, tag="t2b", name="t2b")
                t3 = TMP.tile([128, 512], BF16, tag="t3b", name="t3b")
                nc.vector.tensor_mul(psr[:], psr[:], cos4[:, tsl])
                nc.vector.tensor_mul(t2b[:], psrr[:], sin4[:, tsl])
                nc.vector.tensor_add(t3[:], psr[:], t2b[:])
                for h in range(GH):
                    nc.sync.dma_start(
                        qT[C:128, h, tsl], t3[32 * h:32 * h + 32, :]
                    )

            def phC_head(c, h):
                """attention main for (chunk c, head h): scores+exp+ctx+tree.
                Returns state for phC_fin.  Diagonal key chunks last so their
                mask-multiply stays off the exp->ctx critical path."""
                csl = slice(c * 512, (c + 1) * 512)
                nkc = 4 * (c + 1)
                kcs = list(range(4 * c)) + [4 * c + d for d in range(4)]
                ets = {}
                stack = []  # binary-counter tree: list of (level, tile)

                def emit_score(kc, i):
                    ps = PSS.tile([128, 512], F32, tag="s", name=f"ps{i % 2}")
                    nc.tensor.matmul(
                        ps[:],
                        lhsT=kT[:, h, kc * 128:(kc + 1) * 128],
                        rhs=qT[:, h, csl],
                        start=True, stop=True,
                    )
                    et = ETS.tile([128, 512], BF16, tag="e", name=f"et{i % 6}")
                    nc.scalar.activation(
                        et[:], ps[:], mybir.ActivationFunctionType.Exp
                    )
                    off = kc - 4 * c
                    if off >= 0:
                        nc.vector.tensor_mul(
                            et[:], et[:], mask4[:, off * 512:(off + 1) * 512]
                        )
                    ets[kc] = et

                pctx = PSC.tile([128, 512], F32, tag="c", name="pctx")

                def emit_ctx(kc, i):
                    nc.tensor.matmul(
                        pctx[:],
                        lhsT=vG[:, kc, h * DH:(h + 1) * DH],
                        rhs=ets[kc][:],
                        start=(i == 0), stop=(i == nkc - 1),
                    )
                    # fold into the tree-sum (bf16); level-0 pair adds go to
                    # the otherwise-idle GpSimd, upper levels to DVE
                    carry = ets[kc]
                    lvl = 0
                    while stack and stack[-1][0] == lvl:
                        _, other = stack.pop()
                        dst = ACC.tile([128, 512], BF16, tag="a",
                                       name=f"acc{i % 5}")
                        eng = nc.gpsimd if lvl == 0 else nc.vector
                        eng.tensor_add(dst[:], other[:], carry[:])
                        carry = dst
                        lvl += 1
                    stack.append((lvl, carry))

                LAG = 3
                for i, kc in enumerate(kcs):
                    emit_score(kc, i)
                    if i >= LAG:
                        emit_ctx(kcs[i - LAG], i - LAG)
                for i in range(max(0, nkc - LAG), nkc):
                    emit_ctx(kcs[i], i)
                # fold remaining tree levels
                while len(stack) > 1:
                    l1, a = stack.pop()
                    l2, b = stack.pop()
                    dst = ACC.tile([128, 512], BF16, tag="a", name="accf")
                    nc.vector.tensor_add(dst[:], a[:], b[:])
                    stack.append((max(l1, l2) + 1, dst))
                return pctx, stack[0][1]

            def phC_fin(c, h, pctx, tsum):
                """row-sum via all-ones matmul, reciprocal, ctx normalize."""
                csl = slice(c * 512, (c + 1) * 512)
                prs = PSM.tile([128, 512], F32, tag="m", name="prs")
                nc.tensor.matmul(
                    prs[:], lhsT=onesb[:], rhs=tsum[:], start=True, stop=True
                )
                rc = RCP.tile([128, 512], F32, tag="rc", name="rc")
                nc.vector.reciprocal_approx_fast(out=rc[:], in_=prs[:])
                nc.vector.tensor_mul(ctxT[:, h, csl], pctx[:], rc[:])

            def phC(c):
                for h in range(GH):
                    pctx, tsum = phC_head(c, h)
                    phC_fin(c, h, pctx, tsum)

            def load_wo():
                if WO[0] is None:
                    WO[0] = tc.alloc_tile_pool(name="wo", bufs=1, side="right")
                    wo_sb[0] = WO[0].tile([128, 4, HID], BF16, tag="wo",
                                          name="wo")
                    for hc in range(4):
                        nc.sync.dma_start(wo_sb[0][:, hc, :], woT_r[hc])

            def phD_qb(qb):
                """partial out-projection for one 128-row query block.
                psum evac copies alternate DVE / ACT to split the load."""
                for oc in range(4):
                    ps = PSM.tile([128, 512], F32, tag="m",
                                  name=f"po{oc % 3}")
                    for h in range(GH):
                        nc.tensor.matmul(
                            ps[:],
                            lhsT=ctxT[:, h, qb * 128:(qb + 1) * 128],
                            rhs=wo_sb[0][:, h, oc * 512:(oc + 1) * 512],
                            start=(h == 0), stop=(h == 3),
                        )
                    ot = OT.tile([128, 512], F32, tag="ot", name="ot")
                    if oc % 2 == 0:
                        nc.vector.tensor_copy(ot[:], ps[:])
                    else:
                        nc.scalar.copy(ot[:], ps[:])
                    nc.sync.dma_start(
                        out_d[qb * 128:(qb + 1) * 128,
                              oc * 512:(oc + 1) * 512],
                        ot[:],
                    )

            # ---------------- master schedule ----------------
            xt0 = load_x(0)
            load_wd()
            load_cossin()
            # warm-up matmuls on a memset tile: PE starts at ~+2us (no DMA
            # dependency), so HAM is un-throttled before the real work
            wmt = perB.tile([128, 512], BF16, tag="wmt", name="wmt")
            nc.gpsimd.memset(wmt[:], 0.0)
            for i in range(26):
                pw = PSM.tile([128, 512], F32, tag="m", name=f"warm{i % 2}")
                nc.tensor.matmul(pw[:], lhsT=wmt[:, 0:128], rhs=wmt[:],
                                 start=True, stop=True)
            lat0 = phA(0, xt0)
            load_aux_weights()
            phB_k(0, lat0[0])
            phB_v(0, lat0[0])
            phB_qc(0, lat0[1])
            phB_qr(0, lat0[1])
            lat1 = phA(1)

            def run_chunk(c, fillers):
                for h in range(GH):
                    st = phC_head(c, h)
                    for f in fillers[h]:
                        f()
                    phC_fin(c, h, *st)

            # per-head fillers keep the PE fed while ACT does the exps
            run_chunk(0, [
                [lambda: phB_k(1, lat1[0])],
                [lambda: phB_v(1, lat1[0])],
                [lambda: phB_qc(1, lat1[1])],
                [lambda: phB_qr(1, lat1[1])],
            ])
            lat2 = phA(2)
            run_chunk(1, [
                [lambda: phB_k(2, lat2[0])],
                [lambda: phB_v(2, lat2[0])],
                [lambda: phB_qc(2, lat2[1])],
                [lambda: phB_qr(2, lat2[1])],
            ])
            lat3 = phA(3)
            WD.release()
            load_wo()
            run_chunk(2, [
                [lambda: phB_k(3, lat3[0])],
                [lambda: phB_v(3, lat3[0])],
                [lambda: phB_qc(3, lat3[1])],
                [lambda: phB_qr(3, lat3[1]), lambda: phD_qb(0)],
            ])
            run_chunk(3, [
                [lambda: phD_qb(1), lambda: phD_qb(2)],
                [lambda: phD_qb(3), lambda: phD_qb(4)],
                [lambda: phD_qb(5), lambda: phD_qb(6)],
                [lambda: phD_qb(7), lambda: phD_qb(8)],
            ])
            for qb in range(9, 16):
                phD_qb(qb)
            if WO[0] is not None:
                WO[0].release()

    nc.compile()
    return nc


def _rot_rows(w):
    # rows of w are the rope dim; rot(w) @ lat == rotate_half(w @ lat)
    hR = w.shape[0] // 2
    return np.concatenate([-w[hR:], w[:hR]], axis=0)


def _prep_inputs(inputs):
    x = np.asarray(inputs["x"], np.float32)
    Wq_down = np.asarray(inputs["Wq_down"], np.float32)
    Wq_up = np.asarray(inputs["Wq_up"], np.float32)
    Wq_rope = np.asarray(inputs["Wq_rope"], np.float32)
    Wkv_down = np.asarray(inputs["Wkv_down"], np.float32)
    Wk_up = np.asarray(inputs["Wk_up"], np.float32)
    Wk_rope = np.asarray(inputs["Wk_rope"], np.float32)
    Wv_up = np.asarray(inputs["Wv_up"], np.float32)
    Wo = np.asarray(inputs["Wo"], np.float32)

    s = np.float32(1.0 / np.sqrt(DH))

    wd_kvT = np.ascontiguousarray(Wkv_down.T).astype(BF16NP)
    wd_qT = np.ascontiguousarray(Wq_down.T).astype(BF16NP)
    wkr2 = np.concatenate([Wk_rope, _rot_rows(Wk_rope)], axis=0)  # [64, HID]
    wkr2T = np.ascontiguousarray(wkr2.T).astype(BF16NP)

    inv_freq = (1.0 / (10000.0 ** (np.arange(0, R, 2, dtype=np.float32) / R)))
    t = np.arange(S, dtype=np.float32)
    freqs = t[:, None] * inv_freq[None, :]
    emb = np.concatenate([freqs, freqs], axis=-1)          # [S, R]
    cos4 = np.tile(np.cos(emb).T, (4, 1)).astype(BF16NP)   # [128, S]
    sin4 = np.tile(np.sin(emb).T, (4, 1)).astype(BF16NP)

    kar = np.arange(128)[:, None]
    qar = np.arange(512)[None, :]
    mask4 = np.empty((128, NQC * 512), np.float32)
    for off in range(4):
        mask4[:, off * 512:(off + 1) * 512] = (
            (128 * off + kar) <= qar
        ).astype(np.float32)
    mask4 = mask4.astype(BF16NP)

    per_g = []
    for g in range(4):
        hsl = slice(g * GH, (g + 1) * GH)
        wk_p = np.concatenate(
            [Wk_up[h * C:(h + 1) * C] for h in range(g * GH, (g + 1) * GH)],
            axis=0)                                         # [384, LAT]
        wv_p = np.concatenate(
            [Wv_up[h * DH:(h + 1) * DH] for h in range(g * GH, (g + 1) * GH)],
            axis=0)                                         # [512, LAT]
        wqc_p = np.concatenate(
            [Wq_up[h * C:(h + 1) * C] for h in range(g * GH, (g + 1) * GH)],
            axis=0) * s
        wqr_p = np.concatenate(
            [Wq_rope[h * R:(h + 1) * R] for h in range(g * GH, (g + 1) * GH)],
            axis=0) * s
        wqrr_p = np.concatenate(
            [_rot_rows(Wq_rope[h * R:(h + 1) * R])
             for h in range(g * GH, (g + 1) * GH)], axis=0) * s
        wo_g = Wo[:, g * GH * DH:(g + 1) * GH * DH]         # [HID, 512]
        per_g.append({
            "wk_pT": np.ascontiguousarray(wk_p.T).astype(BF16NP),
            "wv_pT": np.ascontiguousarray(wv_p.T).astype(BF16NP),
            "wqc_pT": np.ascontiguousarray(wqc_p.T).astype(BF16NP),
            "wqr_pT": np.ascontiguousarray(wqr_p.T).astype(BF16NP),
            "wqrr_pT": np.ascontiguousarray(wqrr_p.T).astype(BF16NP),
            "woT": np.ascontiguousarray(wo_g.T).astype(BF16NP),
        })

    in_maps = []
    for cid in range(8):
        b, g = divmod(cid, 4)
        m = {
            "xbT": np.ascontiguousarray(x[b].T).astype(BF16NP),
            "wd_kvT": wd_kvT, "wd_qT": wd_qT, "wkr2T": wkr2T,
            "cos4": cos4, "sin4": sin4, "mask4": mask4,
        }
        m.update(per_g[g])
        in_maps.append(m)
    return in_maps


_NC_CACHE = None


def kernel(**inputs):
    global _NC_CACHE
    if _NC_CACHE is None:
        _NC_CACHE = build_nc()
    nc = _NC_CACHE
    in_maps = _prep_inputs(inputs)
    res = run_bass_kernel_spmd(nc, in_maps, list(range(8)))
    bo = np.asarray(inputs["bo"], np.float32)
    out = np.empty((B, S, HID), np.float32)
    for b in range(B):
        acc = res.results[4 * b]["out"].astype(np.float32)
        for g in range(1, 4):
            acc = acc + res.results[4 * b + g]["out"]
        out[b] = acc + bo
    return out


# revision 53
# speedup vs baseline: 1.0703x; 1.0703x over previous
"""MLA (multi-head latent attention) Trainium2 kernel, SPMD over 8 NeuronCores.

Sharding: core c = 4*b + g handles batch b and head group g (4 heads),
ALL 2048 query rows.  Causality: query chunk c (512 rows) only attends
key chunks 0..4c+3 (lower triangle), so every core does the same
triangular work -- perfectly balanced, no masks off the diagonal.
Each core emits a PARTIAL out-projection (contraction over its 4 heads'
128-dims); the host sums the 4 partials per batch (+bias).  No
collectives.

On-chip layouts are transposed ([feature, token]) so every matmul
contracts over the partition dim with no on-chip transposes.
rotate_half is folded into host-permuted weight copies; 1/sqrt(dh) into
the q weights; softmax skips the max-pass (scores bounded) and gets its
row-sum from an all-ones matmul over a DVE tree-sum of the exp tiles.
Diagonal score tiles are masked multiplicatively (0/1 bf16) after exp.
"""

import os
import sys
import types

for _p in ("/opt/trn_rl_repo", "/root/.axon_site/_ro/trn_rl_repo"):
    if os.path.isdir(_p) and _p not in sys.path:
        sys.path.append(_p)

import numpy as np
import ml_dtypes

import concourse.bass as bass
import concourse.bacc as bacc_mod
import concourse.mybir as mybir
from concourse.tile import TileContext
from concourse.vector_clock import ScopedClock
from concourse.bass_utils import run_bass_kernel_spmd

F32 = mybir.dt.float32
BF16 = mybir.dt.bfloat16
BF16NP = ml_dtypes.bfloat16

HID, H, LAT, R, DH, C = 2048, 16, 512, 32, 128, 96
B, S = 2, 2048
GH = 4            # heads per core
NQC = 4           # query chunks of 512
NKC = 16          # key chunks of 128


def _patch_tile_drain():
    """The staged walrus rejects a Drain carrying >1 sync-wait. Move the
    TileContext tail-drain waits onto single-wait SP nops."""

    def _drain_and_barrier(self, tick_clock, wait_clock):
        drain_inst = self.nc.sync.drain()
        wait_clock.add_sem_waits(
            drain_inst.ins, ScopedClock({None: tick_clock.global_clock})
        )
        si = drain_inst.ins.sync_info
        if si is not None and len(si.on_wait) > 1:
            waits = list(si.on_wait)
            drain_inst.ins.sync_info = mybir.SyncInfo(
                on_wait=[], on_update=list(si.on_update)
            )
            for w in waits:
                nop = self.nc.sync.nop(nofuse=True)
                nop.ins.sync_info = mybir.SyncInfo(on_wait=[w], on_update=[])
        self.nc.all_engine_barrier()
        assert self.sems is not None
        popped = self.nc._tile_sem_poison_stack.pop()
        assert popped is self._sem_poison
        self.nc.clear_and_free_semaphores(list(self.sems.allocated().values()))
        self.nc.all_engine_barrier()

    TileContext._drain_and_barrier = _drain_and_barrier


def _install_ntff_hook():
    """antenv.axon_hooks is absent in this image; inject it and register the
    ctypes NTFF hook so trace=True / BASS_TRACE can profile."""
    try:
        import antenv

        if "antenv.axon_hooks" not in sys.modules:
            mod = types.ModuleType("antenv.axon_hooks")
            mod._hook = None

            def set_axon_ntff_profile_hook(h):
                mod._hook = h

            def get_axon_ntff_profile_hook():
                return mod._hook

            mod.set_axon_ntff_profile_hook = set_axon_ntff_profile_hook
            mod.get_axon_ntff_profile_hook = get_axon_ntff_profile_hook
            sys.modules["antenv.axon_hooks"] = mod
            antenv.axon_hooks = mod
        boot_dir = "/root/.axon_site/trn_agent_boot"
        so_path = "/opt/axon/libaxon_pjrt.so"
        if os.path.isdir(boot_dir) and os.path.exists(so_path):
            if boot_dir not in sys.path:
                sys.path.append(boot_dir)
            from trn_boot import _ntff_profile_via_ctypes

            hook = _ntff_profile_via_ctypes(so_path)
            if hook is not None:
                sys.modules["antenv.axon_hooks"].set_axon_ntff_profile_hook(hook)
    except Exception:
        pass


_patch_tile_drain()
_install_ntff_hook()


def _dram(nc, name, shape, dtype=F32, out=False):
    return nc.declare_dram_parameter(name, list(shape), dtype, isOutput=out)


def build_nc():
    nc = bacc_mod.Bacc("TRN2")

    xbT = _dram(nc, "xbT", [HID, S], BF16)            # x[b].T
    wd_kvT = _dram(nc, "wd_kvT", [HID, LAT], BF16)    # Wkv_down.T
    wd_qT = _dram(nc, "wd_qT", [HID, LAT], BF16)      # Wq_down.T
    wkr2T = _dram(nc, "wkr2T", [HID, 2 * R], BF16)    # [Wk_rope; rot].T
    wk_pT = _dram(nc, "wk_pT", [LAT, GH * C], BF16)   # 4-head k_c pack .T
    wv_pT = _dram(nc, "wv_pT", [LAT, GH * DH], BF16)  # 4-head v pack .T
    wqc_pT = _dram(nc, "wqc_pT", [LAT, GH * C], BF16)   # 4-head q_c pack /sqrt
    wqr_pT = _dram(nc, "wqr_pT", [LAT, GH * R], BF16)   # 4-head q_rope /sqrt
    wqrr_pT = _dram(nc, "wqrr_pT", [LAT, GH * R], BF16)  # rotated rope /sqrt
    woT = _dram(nc, "woT", [GH * DH, HID], BF16)      # Wo cols for our heads
    cos4_d = _dram(nc, "cos4", [128, S], BF16)        # cos.T tiled 4x
    sin4_d = _dram(nc, "sin4", [128, S], BF16)
    mask4_d = _dram(nc, "mask4", [128, NQC * 512], BF16)  # 0/1 diag masks
    out_d = _dram(nc, "out", [S, HID], out=True)      # partial (4-head) proj

    xbT_r = xbT[:, :].rearrange("(c p two) t -> c p two t", p=128, two=2)
    wd_kvT_r = wd_kvT[:, :].rearrange("(c p two) l -> c p two l", p=128, two=2)
    wd_qT_r = wd_qT[:, :].rearrange("(c p two) l -> c p two l", p=128, two=2)
    wkr2T_r = wkr2T[:, :].rearrange("(c p two) r -> c p two r", p=128, two=2)
    wk_pT_r = wk_pT[:, :].rearrange("(lc p) d -> lc p d", p=128)
    wv_pT_r = wv_pT[:, :].rearrange("(lc p) d -> lc p d", p=128)
    wqc_pT_r = wqc_pT[:, :].rearrange("(lc p) d -> lc p d", p=128)
    wqr_pT_r = wqr_pT[:, :].rearrange("(lc p) d -> lc p d", p=128)
    wqrr_pT_r = wqrr_pT[:, :].rearrange("(lc p) d -> lc p d", p=128)
    woT_r = woT[:, :].rearrange("(hc p) o -> hc p o", p=128)

    with TileContext(nc) as tc:
        with tc.tile_pool(name="perB", bufs=1) as perB, \
             tc.tile_pool(name="lat", bufs=2) as LATP, \
             tc.tile_pool(name="xs", bufs=1) as XS, \
             tc.tile_pool(name="ets", bufs=6) as ETS, \
             tc.tile_pool(name="acc", bufs=8) as ACC, \
             tc.tile_pool(name="rcp", bufs=2) as RCP, \
             tc.tile_pool(name="tmp", bufs=2) as TMP, \
             tc.tile_pool(name="ot", bufs=3) as OT, \
             tc.tile_pool(name="ps_g", bufs=2, space="PSUM") as PSG, \
             tc.tile_pool(name="ps_m", bufs=2, space="PSUM") as PSM, \
             tc.tile_pool(name="ps_s", bufs=2, space="PSUM") as PSS, \
             tc.tile_pool(name="ps_c", bufs=2, space="PSUM") as PSC:

            # ---------- persistent SBUF ----------
            krT = perB.tile([32, S], BF16, tag="krT", name="krT")
            kT = perB.tile([128, GH, S], BF16, tag="kT", name="kT")
            vG = perB.tile([128, NKC, GH * DH], BF16, tag="vG", name="vG")
            qT = perB.tile([128, GH, S], BF16, tag="qT", name="qT")
            ctxT = perB.tile([128, GH, S], BF16, tag="ctxT", name="ctxT")
            cos4 = perB.tile([128, S], BF16, tag="cos4", name="cos4")
            sin4 = perB.tile([128, S], BF16, tag="sin4", name="sin4")
            mask4 = perB.tile([128, NQC * 512], BF16, tag="mask4", name="mask4")
            onesb = perB.tile([128, 128], BF16, tag="ones", name="ones")
            wk_sb = perB.tile([128, 4, GH * C], BF16, tag="wk", name="wk")
            wv_sb = perB.tile([128, 4, GH * DH], BF16, tag="wv", name="wv")
            wqc_sb = perB.tile([128, 4, GH * C], BF16, tag="wqc", name="wqc")
            wqr_sb = perB.tile([128, 4, GH * R], BF16, tag="wqr", name="wqr")
            wqrr_sb = perB.tile([128, 4, GH * R], BF16, tag="wqrr", name="wqrr")

            # down-proj weights: released after phase A(3), wo loaded after.
            # Per-hc tiles so the first matmuls wait only on their own slice;
            # kv weights first (the very first accumulation pass).
            WD = tc.alloc_tile_pool(name="wd", bufs=1, side="right")
            wdkv = [WD.tile([128, 2, LAT], BF16, tag=f"wdkv{hc}",
                            name=f"wdkv{hc}") for hc in range(8)]
            wdq = [WD.tile([128, 2, LAT], BF16, tag=f"wdq{hc}",
                           name=f"wdq{hc}") for hc in range(8)]
            wkr = [WD.tile([128, 2, 2 * R], BF16, tag=f"wkr{hc}",
                           name=f"wkr{hc}") for hc in range(8)]

            def load_wd():
                for hc in range(8):
                    nc.sync.dma_start(wdkv[hc][:], wd_kvT_r[hc])
                for hc in range(8):
                    nc.sync.dma_start(wkr[hc][:], wkr2T_r[hc])
                    nc.sync.dma_start(wdq[hc][:], wd_qT_r[hc])

            def load_x(tq):
                tsl = slice(tq * 512, (tq + 1) * 512)
                xt = [XS.tile([128, 2, 512], BF16, tag=f"xf{hc}",
                              name=f"xf{hc}") for hc in range(8)]
                for hc in range(8):
                    nc.sync.dma_start(xt[hc][:], xbT_r[hc][:, :, tsl])
                return xt

            WO = [None]  # box for the late wo pool
            wo_sb = [None]

            def load_cossin():
                nc.sync.dma_start(cos4[:], cos4_d[:, :])
                nc.sync.dma_start(sin4[:], sin4_d[:, :])

            def load_aux_weights():
                nc.sync.dma_start(mask4[:], mask4_d[:, :])
                nc.gpsimd.memset(onesb[:], 1.0)
                for lc in range(4):
                    nc.sync.dma_start(wk_sb[:, lc, :], wk_pT_r[lc])
                    nc.sync.dma_start(wv_sb[:, lc, :], wv_pT_r[lc])
                    nc.sync.dma_start(wqc_sb[:, lc, :], wqc_pT_r[lc])
                    nc.sync.dma_start(wqr_sb[:, lc, :], wqr_pT_r[lc])
                    nc.sync.dma_start(wqrr_sb[:, lc, :], wqrr_pT_r[lc])

            # ---------------- phase emitters ----------------
            def phA(tq, xt=None):
                """latents for token quarter tq: kv_lat, roped k_rope, q_lat.
                Returns the per-quarter latent tiles for phB(tq)."""
                tsl = slice(tq * 512, (tq + 1) * 512)
                if xt is None:
                    xt = load_x(tq)
                kv_t = LATP.tile([128, 4, 512], BF16, tag="kvlat",
                                 name="kvlat")
                q_t = LATP.tile([128, 4, 512], BF16, tag="qlat", name="qlat")

                # kv_lat: 4 lc passes, 2 rotating psum banks
                for lc in range(4):
                    ps = PSG.tile([128, 512], F32, tag="g", name=f"pkv{lc}")
                    for hc in range(8):
                        for two in range(2):
                            nc.tensor.matmul(
                                ps[:],
                                lhsT=wdkv[hc][:, two, lc * 128:(lc + 1) * 128],
                                rhs=xt[hc][:, two, :],
                                start=(hc == 0 and two == 0),
                                stop=(hc == 7 and two == 1),
                            )
                    nc.vector.tensor_copy(kv_t[:, lc, :], ps[:])
                # k_rope pass (64 rows: [rope; rot]); combine in place
                pkr = PSG.tile([64, 512], F32, tag="g", name="pkr")
                for hc in range(8):
                    for two in range(2):
                        nc.tensor.matmul(
                            pkr[:],
                            lhsT=wkr[hc][:, two, :],
                            rhs=xt[hc][:, two, :],
                            start=(hc == 0 and two == 0),
                            stop=(hc == 7 and two == 1),
                        )
                nc.vector.tensor_mul(pkr[0:32, :], pkr[0:32, :],
                                     cos4[0:32, tsl])
                tkr = TMP.tile([32, 512], F32, tag="tkr", name="tkr")
                nc.vector.tensor_mul(tkr[:], pkr[32:64, :], sin4[0:32, tsl])
                nc.vector.tensor_add(krT[:, tsl], pkr[0:32, :], tkr[:])
                # q_lat: 4 lc passes
                for lc in range(4):
                    ps = PSG.tile([128, 512], F32, tag="g", name=f"pq{lc}")
                    for hc in range(8):
                        for two in range(2):
                            nc.tensor.matmul(
                                ps[:],
                                lhsT=wdq[hc][:, two, lc * 128:(lc + 1) * 128],
                                rhs=xt[hc][:, two, :],
                                start=(hc == 0 and two == 0),
                                stop=(hc == 7 and two == 1),
                            )
                    nc.vector.tensor_copy(q_t[:, lc, :], ps[:])
                return kv_t, q_t

            def phB_k(tq, kv_t):
                """k_c per head (96 content rows) + shared roped k_rope."""
                tsl = slice(tq * 512, (tq + 1) * 512)
                for h in range(GH):
                    ps = PSM.tile([128, 512], F32, tag="m", name=f"pk{h}")
                    for lc in range(4):
                        nc.tensor.matmul(
                            ps[0:C, :],
                            lhsT=wk_sb[:, lc, h * C:(h + 1) * C],
                            rhs=kv_t[:, lc, :],
                            start=(lc == 0), stop=(lc == 3),
                        )
                    nc.vector.tensor_copy(kT[0:C, h, tsl], ps[0:C, :])
                for h in range(GH):
                    nc.sync.dma_start(kT[C:128, h, tsl], krT[:, tsl])

            def phB_v(tq, kv_t):
                """v: 4 token sub-chunks of 128, out = [t, 4h*128]."""
                for t2 in range(4):
                    kc = tq * 4 + t2
                    ps = PSM.tile([128, 512], F32, tag="m", name=f"pv{t2}")
                    for lc in range(4):
                        nc.tensor.matmul(
                            ps[:],
                            lhsT=kv_t[:, lc, t2 * 128:(t2 + 1) * 128],
                            rhs=wv_sb[:, lc, :],
                            start=(lc == 0), stop=(lc == 3),
                        )
                    nc.vector.tensor_copy(vG[:, kc, :], ps[:])

            def phB_qc(tq, q_t):
                """q_c per head."""
                tsl = slice(tq * 512, (tq + 1) * 512)
                for h in range(GH):
                    ps = PSM.tile([128, 512], F32, tag="m", name=f"pqc{h}")
                    for lc in range(4):
                        nc.tensor.matmul(
                            ps[0:C, :],
                            lhsT=wqc_sb[:, lc, h * C:(h + 1) * C],
                            rhs=q_t[:, lc, :],
                            start=(lc == 0), stop=(lc == 3),
                        )
                    nc.vector.tensor_copy(qT[0:C, h, tsl], ps[0:C, :])

            def phB_qr(tq, q_t):
                """q_rope: stacked 4h x 32 rope + rot; combine, scatter."""
                tsl = slice(tq * 512, (tq + 1) * 512)
                psr = PSM.tile([128, 512], F32, tag="m", name="pqr")
                psrr = PSM.tile([128, 512], F32, tag="m", name="pqrr")
                for lc in range(4):
                    nc.tensor.matmul(
                        psr[:], lhsT=wqr_sb[:, lc, :],
                        rhs=q_t[:, lc, :],
                        start=(lc == 0), stop=(lc == 3),
                    )
                for lc in range(4):
                    nc.tensor.matmul(
                        psrr[:], lhsT=wqrr_sb[:, lc, :],
                        rhs=q_t[:, lc, :],
                        start=(lc == 0), stop=(lc == 3),
                    )
                t2b = TMP.tile([128, 512], F32, tag="t2b", name="t2b")
                t3 = TMP.tile([128, 512], BF16, tag="t3b", name="t3b")
                nc.vector.tensor_mul(psr[:], psr[:], cos4[:, tsl])
                nc.vector.tensor_mul(t2b[:], psrr[:], sin4[:, tsl])
                nc.vector.tensor_add(t3[:], psr[:], t2b[:])
                for h in range(GH):
                    nc.sync.dma_start(
                        qT[C:128, h, tsl], t3[32 * h:32 * h + 32, :]
                    )

            def phC_head(c, h):
                """attention main for (chunk c, head h): scores+exp+ctx+tree.
                Returns state for phC_fin.  Diagonal key chunks last so their
                mask-multiply stays off the exp->ctx critical path."""
                csl = slice(c * 512, (c + 1) * 512)
                nkc = 4 * (c + 1)
                kcs = list(range(4 * c)) + [4 * c + d for d in range(4)]
                ets = {}
                stack = []  # binary-counter tree: list of (level, tile)

                def emit_score(kc, i):
                    ps = PSS.tile([128, 512], F32, tag="s", name=f"ps{i % 2}")
                    nc.tensor.matmul(
                        ps[:],
                        lhsT=kT[:, h, kc * 128:(kc + 1) * 128],
                        rhs=qT[:, h, csl],
                        start=True, stop=True,
                    )
                    et = ETS.tile([128, 512], BF16, tag="e", name=f"et{i % 6}")
                    nc.scalar.activation(
                        et[:], ps[:], mybir.ActivationFunctionType.Exp
                    )
                    off = kc - 4 * c
                    if off >= 0:
                        nc.vector.tensor_mul(
                            et[:], et[:], mask4[:, off * 512:(off + 1) * 512]
                        )
                    ets[kc] = et

                pctx = PSC.tile([128, 512], F32, tag="c", name="pctx")

                def emit_ctx(kc, i):
                    nc.tensor.matmul(
                        pctx[:],
                        lhsT=vG[:, kc, h * DH:(h + 1) * DH],
                        rhs=ets[kc][:],
                        start=(i == 0), stop=(i == nkc - 1),
                    )
                    # fold into the tree-sum (bf16); level-0 pair adds go to
                    # the otherwise-idle GpSimd, upper levels to DVE
                    carry = ets[kc]
                    lvl = 0
                    while stack and stack[-1][0] == lvl:
                        _, other = stack.pop()
                        dst = ACC.tile([128, 512], BF16, tag="a",
                                       name=f"acc{i % 5}")
                        nc.vector.tensor_add(dst[:], other[:], carry[:])
                        carry = dst
                        lvl += 1
                    stack.append((lvl, carry))

                LAG = 3
                for i, kc in enumerate(kcs):
                    emit_score(kc, i)
                    if i >= LAG:
                        emit_ctx(kcs[i - LAG], i - LAG)
                for i in range(max(0, nkc - LAG), nkc):
                    emit_ctx(kcs[i], i)
                # fold remaining tree levels
                while len(stack) > 1:
                    l1, a = stack.pop()
                    l2, b = stack.pop()
                    dst = ACC.tile([128, 512], BF16, tag="a", name="accf")
                    nc.vector.tensor_add(dst[:], a[:], b[:])
                    stack.append((max(l1, l2) + 1, dst))
                return pctx, stack[0][1]

            def phC_fin(c, h, pctx, tsum):
                """row-sum via all-ones matmul, reciprocal, ctx normalize."""
                csl = slice(c * 512, (c + 1) * 512)
                prs = PSM.tile([128, 512], F32, tag="m", name="prs")
                nc.tensor.matmul(
                    prs[:], lhsT=onesb[:], rhs=tsum[:], start=True, stop=True
                )
                rc = RCP.tile([128, 512], F32, tag="rc", name="rc")
                nc.vector.reciprocal_approx_fast(out=rc[:], in_=prs[:])
                nc.vector.tensor_mul(ctxT[:, h, csl], pctx[:], rc[:])

            def phC(c):
                for h in range(GH):
                    pctx, tsum = phC_head(c, h)
                    phC_fin(c, h, pctx, tsum)

            def load_wo():
                if WO[0] is None:
                    WO[0] = tc.alloc_tile_pool(name="wo", bufs=1, side="right")
                    wo_sb[0] = WO[0].tile([128, 4, HID], BF16, tag="wo",
                                          name="wo")
                    for hc in range(4):
                        nc.sync.dma_start(wo_sb[0][:, hc, :], woT_r[hc])

            def phD_qb(qb):
                """partial out-projection for one 128-row query block.
                psum evac copies alternate DVE / ACT to split the load."""
                for oc in range(4):
                    ps = PSM.tile([128, 512], F32, tag="m",
                                  name=f"po{oc % 3}")
                    for h in range(GH):
                        nc.tensor.matmul(
                            ps[:],
                            lhsT=ctxT[:, h, qb * 128:(qb + 1) * 128],
                            rhs=wo_sb[0][:, h, oc * 512:(oc + 1) * 512],
                            start=(h == 0), stop=(h == 3),
                        )
                    ot = OT.tile([128, 512], F32, tag="ot", name="ot")
                    nc.vector.tensor_copy(ot[:], ps[:])
                    nc.sync.dma_start(
                        out_d[qb * 128:(qb + 1) * 128,
                              oc * 512:(oc + 1) * 512],
                        ot[:],
                    )

            # ---------------- master schedule ----------------
            xt0 = load_x(0)
            load_wd()
            load_cossin()
            # warm-up matmuls on a memset tile: PE starts at ~+2us (no DMA
            # dependency), so HAM is un-throttled before the real work
            wmt = perB.tile([128, 512], BF16, tag="wmt", name="wmt")
            nc.gpsimd.memset(wmt[:], 0.0)
            for i in range(26):
                pw = PSM.tile([128, 512], F32, tag="m", name=f"warm{i % 2}")
                nc.tensor.matmul(pw[:], lhsT=wmt[:, 0:128], rhs=wmt[:],
                                 start=True, stop=True)
            lat0 = phA(0, xt0)
            load_aux_weights()
            phB_k(0, lat0[0])
            phB_v(0, lat0[0])
            phB_qc(0, lat0[1])
            phB_qr(0, lat0[1])
            lat1 = phA(1)

            def run_chunk(c, fillers):
                for h in range(GH):
                    st = phC_head(c, h)
                    for f in fillers[h]:
                        f()
                    phC_fin(c, h, *st)

            # per-head fillers keep the PE fed while ACT does the exps
            run_chunk(0, [
                [lambda: phB_k(1, lat1[0])],
                [lambda: phB_v(1, lat1[0])],
                [lambda: phB_qc(1, lat1[1])],
                [lambda: phB_qr(1, lat1[1])],
            ])
            lat2 = phA(2)
            run_chunk(1, [
                [lambda: phB_k(2, lat2[0])],
                [lambda: phB_v(2, lat2[0])],
                [lambda: phB_qc(2, lat2[1])],
                [lambda: phB_qr(2, lat2[1])],
            ])
            lat3 = phA(3)
            WD.release()
            load_wo()
            run_chunk(2, [
                [lambda: phB_k(3, lat3[0])],
                [lambda: phB_v(3, lat3[0])],
                [lambda: phB_qc(3, lat3[1])],
                [lambda: phB_qr(3, lat3[1]), lambda: phD_qb(0)],
            ])
            run_chunk(3, [
                [lambda: phD_qb(1), lambda: phD_qb(2)],
                [lambda: phD_qb(3), lambda: phD_qb(4)],
                [lambda: phD_qb(5), lambda: phD_qb(6)],
                [lambda: phD_qb(7), lambda: phD_qb(8)],
            ])
            for qb in range(9, 16):
                phD_qb(qb)
            if WO[0] is not None:
                WO[0].release()

    nc.compile()
    return nc


def _rot_rows(w):
    # rows of w are the rope dim; rot(w) @ lat == rotate_half(w @ lat)
    hR = w.shape[0] // 2
    return np.concatenate([-w[hR:], w[:hR]], axis=0)


def _prep_inputs(inputs):
    x = np.asarray(inputs["x"], np.float32)
    Wq_down = np.asarray(inputs["Wq_down"], np.float32)
    Wq_up = np.asarray(inputs["Wq_up"], np.float32)
    Wq_rope = np.asarray(inputs["Wq_rope"], np.float32)
    Wkv_down = np.asarray(inputs["Wkv_down"], np.float32)
    Wk_up = np.asarray(inputs["Wk_up"], np.float32)
    Wk_rope = np.asarray(inputs["Wk_rope"], np.float32)
    Wv_up = np.asarray(inputs["Wv_up"], np.float32)
    Wo = np.asarray(inputs["Wo"], np.float32)

    s = np.float32(1.0 / np.sqrt(DH))

    wd_kvT = np.ascontiguousarray(Wkv_down.T).astype(BF16NP)
    wd_qT = np.ascontiguousarray(Wq_down.T).astype(BF16NP)
    wkr2 = np.concatenate([Wk_rope, _rot_rows(Wk_rope)], axis=0)  # [64, HID]
    wkr2T = np.ascontiguousarray(wkr2.T).astype(BF16NP)

    inv_freq = (1.0 / (10000.0 ** (np.arange(0, R, 2, dtype=np.float32) / R)))
    t = np.arange(S, dtype=np.float32)
    freqs = t[:, None] * inv_freq[None, :]
    emb = np.concatenate([freqs, freqs], axis=-1)          # [S, R]
    cos4 = np.tile(np.cos(emb).T, (4, 1)).astype(BF16NP)   # [128, S]
    sin4 = np.tile(np.sin(emb).T, (4, 1)).astype(BF16NP)

    kar = np.arange(128)[:, None]
    qar = np.arange(512)[None, :]
    mask4 = np.empty((128, NQC * 512), np.float32)
    for off in range(4):
        mask4[:, off * 512:(off + 1) * 512] = (
            (128 * off + kar) <= qar
        ).astype(np.float32)
    mask4 = mask4.astype(BF16NP)

    per_g = []
    for g in range(4):
        hsl = slice(g * GH, (g + 1) * GH)
        wk_p = np.concatenate(
            [Wk_up[h * C:(h + 1) * C] for h in range(g * GH, (g + 1) * GH)],
            axis=0)                                         # [384, LAT]
        wv_p = np.concatenate(
            [Wv_up[h * DH:(h + 1) * DH] for h in range(g * GH, (g + 1) * GH)],
            axis=0)                                         # [512, LAT]
        wqc_p = np.concatenate(
            [Wq_up[h * C:(h + 1) * C] for h in range(g * GH, (g + 1) * GH)],
            axis=0) * s
        wqr_p = np.concatenate(
            [Wq_rope[h * R:(h + 1) * R] for h in range(g * GH, (g + 1) * GH)],
            axis=0) * s
        wqrr_p = np.concatenate(
            [_rot_rows(Wq_rope[h * R:(h + 1) * R])
             for h in range(g * GH, (g + 1) * GH)], axis=0) * s
        wo_g = Wo[:, g * GH * DH:(g + 1) * GH * DH]         # [HID, 512]
        per_g.append({
            "wk_pT": np.ascontiguousarray(wk_p.T).astype(BF16NP),
            "wv_pT": np.ascontiguousarray(wv_p.T).astype(BF16NP),
            "wqc_pT": np.ascontiguousarray(wqc_p.T).astype(BF16NP),
            "wqr_pT": np.ascontiguousarray(wqr_p.T).astype(BF16NP),
            "wqrr_pT": np.ascontiguousarray(wqrr_p.T).astype(BF16NP),
            "woT": np.ascontiguousarray(wo_g.T).astype(BF16NP),
        })

    in_maps = []
    for cid in range(8):
        b, g = divmod(cid, 4)
        m = {
            "xbT": np.ascontiguousarray(x[b].T).astype(BF16NP),
            "wd_kvT": wd_kvT, "wd_qT": wd_qT, "wkr2T": wkr2T,
            "cos4": cos4, "sin4": sin4, "mask4": mask4,
        }
        m.update(per_g[g])
        in_maps.append(m)
    return in_maps


_NC_CACHE = None


def kernel(**inputs):
    global _NC_CACHE
    if _NC_CACHE is None:
        _NC_CACHE = build_nc()
    nc = _NC_CACHE
    in_maps = _prep_inputs(inputs)
    res = run_bass_kernel_spmd(nc, in_maps, list(range(8)))
    bo = np.asarray(inputs["bo"], np.float32)
    out = np.empty((B, S, HID), np.float32)
    for b in range(B):
        acc = res.results[4 * b]["out"].astype(np.float32)
        for g in range(1, 4):
            acc = acc + res.results[4 * b + g]["out"]
        out[b] = acc + bo
    return out


# revision 58
# speedup vs baseline: 1.0737x; 1.0032x over previous
"""MLA (multi-head latent attention) Trainium2 kernel, SPMD over 8 NeuronCores.

Sharding: core c = 4*b + g handles batch b and head group g (4 heads),
ALL 2048 query rows.  Causality: query chunk c (512 rows) only attends
key chunks 0..4c+3 (lower triangle), so every core does the same
triangular work -- perfectly balanced, no masks off the diagonal.
Each core emits a PARTIAL out-projection (contraction over its 4 heads'
128-dims); the host sums the 4 partials per batch (+bias).  No
collectives.

On-chip layouts are transposed ([feature, token]) so every matmul
contracts over the partition dim with no on-chip transposes.
rotate_half is folded into host-permuted weight copies; 1/sqrt(dh) into
the q weights; softmax skips the max-pass (scores bounded) and gets its
row-sum from an all-ones matmul over a DVE tree-sum of the exp tiles.
Diagonal score tiles are masked multiplicatively (0/1 bf16) after exp.
"""

import os
import sys
import types

for _p in ("/opt/trn_rl_repo", "/root/.axon_site/_ro/trn_rl_repo"):
    if os.path.isdir(_p) and _p not in sys.path:
        sys.path.append(_p)

import numpy as np
import ml_dtypes

import concourse.bass as bass
import concourse.bacc as bacc_mod
import concourse.mybir as mybir
from concourse.tile import TileContext
from concourse.vector_clock import ScopedClock
from concourse.bass_utils import run_bass_kernel_spmd

F32 = mybir.dt.float32
BF16 = mybir.dt.bfloat16
BF16NP = ml_dtypes.bfloat16

HID, H, LAT, R, DH, C = 2048, 16, 512, 32, 128, 96
B, S = 2, 2048
GH = 4            # heads per core
NQC = 4           # query chunks of 512
NKC = 16          # key chunks of 128


def _patch_tile_drain():
    """The staged walrus rejects a Drain carrying >1 sync-wait. Move the
    TileContext tail-drain waits onto single-wait SP nops."""

    def _drain_and_barrier(self, tick_clock, wait_clock):
        drain_inst = self.nc.sync.drain()
        wait_clock.add_sem_waits(
            drain_inst.ins, ScopedClock({None: tick_clock.global_clock})
        )
        si = drain_inst.ins.sync_info
        if si is not None and len(si.on_wait) > 1:
            waits = list(si.on_wait)
            drain_inst.ins.sync_info = mybir.SyncInfo(
                on_wait=[], on_update=list(si.on_update)
            )
            for w in waits:
                nop = self.nc.sync.nop(nofuse=True)
                nop.ins.sync_info = mybir.SyncInfo(on_wait=[w], on_update=[])
        self.nc.all_engine_barrier()
        assert self.sems is not None
        popped = self.nc._tile_sem_poison_stack.pop()
        assert popped is self._sem_poison
        self.nc.clear_and_free_semaphores(list(self.sems.allocated().values()))
        self.nc.all_engine_barrier()

    TileContext._drain_and_barrier = _drain_and_barrier


def _install_ntff_hook():
    """antenv.axon_hooks is absent in this image; inject it and register the
    ctypes NTFF hook so trace=True / BASS_TRACE can profile."""
    try:
        import antenv

        if "antenv.axon_hooks" not in sys.modules:
            mod = types.ModuleType("antenv.axon_hooks")
            mod._hook = None

            def set_axon_ntff_profile_hook(h):
                mod._hook = h

            def get_axon_ntff_profile_hook():
                return mod._hook

            mod.set_axon_ntff_profile_hook = set_axon_ntff_profile_hook
            mod.get_axon_ntff_profile_hook = get_axon_ntff_profile_hook
            sys.modules["antenv.axon_hooks"] = mod
            antenv.axon_hooks = mod
        boot_dir = "/root/.axon_site/trn_agent_boot"
        so_path = "/opt/axon/libaxon_pjrt.so"
        if os.path.isdir(boot_dir) and os.path.exists(so_path):
            if boot_dir not in sys.path:
                sys.path.append(boot_dir)
            from trn_boot import _ntff_profile_via_ctypes

            hook = _ntff_profile_via_ctypes(so_path)
            if hook is not None:
                sys.modules["antenv.axon_hooks"].set_axon_ntff_profile_hook(hook)
    except Exception:
        pass


_patch_tile_drain()
_install_ntff_hook()


def _dram(nc, name, shape, dtype=F32, out=False):
    return nc.declare_dram_parameter(name, list(shape), dtype, isOutput=out)


def build_nc():
    nc = bacc_mod.Bacc("TRN2")

    xbT = _dram(nc, "xbT", [HID, S], BF16)            # x[b].T
    wd_kvT = _dram(nc, "wd_kvT", [HID, LAT], BF16)    # Wkv_down.T
    wd_qT = _dram(nc, "wd_qT", [HID, LAT], BF16)      # Wq_down.T
    wkr2T = _dram(nc, "wkr2T", [HID, 2 * R], BF16)    # [Wk_rope; rot].T
    wk_pT = _dram(nc, "wk_pT", [LAT, GH * C], BF16)   # 4-head k_c pack .T
    wv_pT = _dram(nc, "wv_pT", [LAT, GH * DH], BF16)  # 4-head v pack .T
    wqc_pT = _dram(nc, "wqc_pT", [LAT, GH * C], BF16)   # 4-head q_c pack /sqrt
    wqr_pT = _dram(nc, "wqr_pT", [LAT, GH * R], BF16)   # 4-head q_rope /sqrt
    wqrr_pT = _dram(nc, "wqrr_pT", [LAT, GH * R], BF16)  # rotated rope /sqrt
    woT = _dram(nc, "woT", [GH * DH, HID], BF16)      # Wo cols for our heads
    cos4_d = _dram(nc, "cos4", [128, S], BF16)        # cos.T tiled 4x
    sin4_d = _dram(nc, "sin4", [128, S], BF16)
    mask4_d = _dram(nc, "mask4", [128, NQC * 512], BF16)  # 0/1 diag masks
    out_d = _dram(nc, "out", [S, HID], out=True)      # partial (4-head) proj

    xbT_r = xbT[:, :].rearrange("(c p two) t -> c p two t", p=128, two=2)
    wd_kvT_r = wd_kvT[:, :].rearrange("(c p two) l -> c p two l", p=128, two=2)
    wd_qT_r = wd_qT[:, :].rearrange("(c p two) l -> c p two l", p=128, two=2)
    wkr2T_r = wkr2T[:, :].rearrange("(c p two) r -> c p two r", p=128, two=2)
    wk_pT_r = wk_pT[:, :].rearrange("(lc p) d -> lc p d", p=128)
    wv_pT_r = wv_pT[:, :].rearrange("(lc p) d -> lc p d", p=128)
    wqc_pT_r = wqc_pT[:, :].rearrange("(lc p) d -> lc p d", p=128)
    wqr_pT_r = wqr_pT[:, :].rearrange("(lc p) d -> lc p d", p=128)
    wqrr_pT_r = wqrr_pT[:, :].rearrange("(lc p) d -> lc p d", p=128)
    woT_r = woT[:, :].rearrange("(hc p) o -> hc p o", p=128)

    with TileContext(nc) as tc:
        with tc.tile_pool(name="perB", bufs=1) as perB, \
             tc.tile_pool(name="lat", bufs=2) as LATP, \
             tc.tile_pool(name="xs", bufs=1) as XS, \
             tc.tile_pool(name="ets", bufs=6) as ETS, \
             tc.tile_pool(name="acc", bufs=8) as ACC, \
             tc.tile_pool(name="rcp", bufs=2) as RCP, \
             tc.tile_pool(name="tmp", bufs=2) as TMP, \
             tc.tile_pool(name="ot", bufs=3) as OT, \
             tc.tile_pool(name="ps_g", bufs=2, space="PSUM") as PSG, \
             tc.tile_pool(name="ps_m", bufs=2, space="PSUM") as PSM, \
             tc.tile_pool(name="ps_s", bufs=2, space="PSUM") as PSS, \
             tc.tile_pool(name="ps_c", bufs=2, space="PSUM") as PSC:

            # ---------- persistent SBUF ----------
            krT = perB.tile([32, S], BF16, tag="krT", name="krT")
            kT = perB.tile([128, GH, S], BF16, tag="kT", name="kT")
            vG = perB.tile([128, NKC, GH * DH], BF16, tag="vG", name="vG")
            qT = perB.tile([128, GH, S], BF16, tag="qT", name="qT")
            ctxT = perB.tile([128, GH, S], BF16, tag="ctxT", name="ctxT")
            cos4 = perB.tile([128, S], BF16, tag="cos4", name="cos4")
            sin4 = perB.tile([128, S], BF16, tag="sin4", name="sin4")
            mask4 = perB.tile([128, NQC * 512], BF16, tag="mask4", name="mask4")
            onesb = perB.tile([128, 128], BF16, tag="ones", name="ones")
            wk_sb = perB.tile([128, 4, GH * C], BF16, tag="wk", name="wk")
            wv_sb = perB.tile([128, 4, GH * DH], BF16, tag="wv", name="wv")
            wqc_sb = perB.tile([128, 4, GH * C], BF16, tag="wqc", name="wqc")
            wqr_sb = perB.tile([128, 4, GH * R], BF16, tag="wqr", name="wqr")
            wqrr_sb = perB.tile([128, 4, GH * R], BF16, tag="wqrr", name="wqrr")

            # down-proj weights: released after phase A(3), wo loaded after.
            # Per-hc tiles so the first matmuls wait only on their own slice;
            # kv weights first (the very first accumulation pass).
            WD = tc.alloc_tile_pool(name="wd", bufs=1, side="right")
            wdkv = [WD.tile([128, 2, LAT], BF16, tag=f"wdkv{hc}",
                            name=f"wdkv{hc}") for hc in range(8)]
            wdq = [WD.tile([128, 2, LAT], BF16, tag=f"wdq{hc}",
                           name=f"wdq{hc}") for hc in range(8)]
            wkr = [WD.tile([128, 2, 2 * R], BF16, tag=f"wkr{hc}",
                           name=f"wkr{hc}") for hc in range(8)]

            def load_wd():
                for hc in range(8):
                    nc.sync.dma_start(wdkv[hc][:], wd_kvT_r[hc])
                for hc in range(8):
                    nc.sync.dma_start(wkr[hc][:], wkr2T_r[hc])
                    nc.sync.dma_start(wdq[hc][:], wd_qT_r[hc])

            def load_x(tq):
                tsl = slice(tq * 512, (tq + 1) * 512)
                xt = [XS.tile([128, 2, 512], BF16, tag=f"xf{hc}",
                              name=f"xf{hc}") for hc in range(8)]
                for hc in range(8):
                    nc.sync.dma_start(xt[hc][:], xbT_r[hc][:, :, tsl])
                return xt

            WO = [None]  # box for the late wo pool
            wo_sb = [None]

            def load_cossin():
                nc.sync.dma_start(cos4[:], cos4_d[:, :])
                nc.sync.dma_start(sin4[:], sin4_d[:, :])

            def load_aux_weights():
                nc.sync.dma_start(mask4[:], mask4_d[:, :])
                nc.gpsimd.memset(onesb[:], 1.0)
                for lc in range(4):
                    nc.sync.dma_start(wk_sb[:, lc, :], wk_pT_r[lc])
                    nc.sync.dma_start(wv_sb[:, lc, :], wv_pT_r[lc])
                    nc.sync.dma_start(wqc_sb[:, lc, :], wqc_pT_r[lc])
                    nc.sync.dma_start(wqr_sb[:, lc, :], wqr_pT_r[lc])
                    nc.sync.dma_start(wqrr_sb[:, lc, :], wqrr_pT_r[lc])

            # ---------------- phase emitters ----------------
            def phA(tq, xt=None):
                """latents for token quarter tq: kv_lat, roped k_rope, q_lat.
                Returns the per-quarter latent tiles for phB(tq)."""
                tsl = slice(tq * 512, (tq + 1) * 512)
                if xt is None:
                    xt = load_x(tq)
                kv_t = LATP.tile([128, 4, 512], BF16, tag="kvlat",
                                 name="kvlat")
                q_t = LATP.tile([128, 4, 512], BF16, tag="qlat", name="qlat")

                # kv_lat: 4 lc passes, 2 rotating psum banks
                for lc in range(4):
                    ps = PSG.tile([128, 512], F32, tag="g", name=f"pkv{lc}")
                    for hc in range(8):
                        for two in range(2):
                            nc.tensor.matmul(
                                ps[:],
                                lhsT=wdkv[hc][:, two, lc * 128:(lc + 1) * 128],
                                rhs=xt[hc][:, two, :],
                                start=(hc == 0 and two == 0),
                                stop=(hc == 7 and two == 1),
                            )
                    nc.vector.tensor_copy(kv_t[:, lc, :], ps[:])
                # k_rope pass (64 rows: [rope; rot]); combine in place
                pkr = PSG.tile([64, 512], F32, tag="g", name="pkr")
                for hc in range(8):
                    for two in range(2):
                        nc.tensor.matmul(
                            pkr[:],
                            lhsT=wkr[hc][:, two, :],
                            rhs=xt[hc][:, two, :],
                            start=(hc == 0 and two == 0),
                            stop=(hc == 7 and two == 1),
                        )
                nc.vector.tensor_mul(pkr[0:32, :], pkr[0:32, :],
                                     cos4[0:32, tsl])
                tkr = TMP.tile([32, 512], F32, tag="tkr", name="tkr")
                nc.vector.tensor_mul(tkr[:], pkr[32:64, :], sin4[0:32, tsl])
                nc.vector.tensor_add(krT[:, tsl], pkr[0:32, :], tkr[:])
                # q_lat: 4 lc passes
                for lc in range(4):
                    ps = PSG.tile([128, 512], F32, tag="g", name=f"pq{lc}")
                    for hc in range(8):
                        for two in range(2):
                            nc.tensor.matmul(
                                ps[:],
                                lhsT=wdq[hc][:, two, lc * 128:(lc + 1) * 128],
                                rhs=xt[hc][:, two, :],
                                start=(hc == 0 and two == 0),
                                stop=(hc == 7 and two == 1),
                            )
                    nc.vector.tensor_copy(q_t[:, lc, :], ps[:])
                return kv_t, q_t

            def phB_gen(tq, kv_t, q_t):
                """per-head projections for quarter tq, as a generator of
                ~0.9us PE pieces (one psum group each)."""
                tsl = slice(tq * 512, (tq + 1) * 512)
                # k_c per head (96 content rows)
                for h in range(GH):
                    ps = PSM.tile([128, 512], F32, tag="m", name=f"pk{h}")
                    for lc in range(4):
                        nc.tensor.matmul(
                            ps[0:C, :],
                            lhsT=wk_sb[:, lc, h * C:(h + 1) * C],
                            rhs=kv_t[:, lc, :],
                            start=(lc == 0), stop=(lc == 3),
                        )
                    nc.vector.tensor_copy(kT[0:C, h, tsl], ps[0:C, :])
                    nc.sync.dma_start(kT[C:128, h, tsl], krT[:, tsl])
                    yield
                # v: 4 token sub-chunks of 128, out = [t, 4h*128]
                for t2 in range(4):
                    kc = tq * 4 + t2
                    ps = PSM.tile([128, 512], F32, tag="m", name=f"pv{t2}")
                    for lc in range(4):
                        nc.tensor.matmul(
                            ps[:],
                            lhsT=kv_t[:, lc, t2 * 128:(t2 + 1) * 128],
                            rhs=wv_sb[:, lc, :],
                            start=(lc == 0), stop=(lc == 3),
                        )
                    nc.vector.tensor_copy(vG[:, kc, :], ps[:])
                    yield
                # q_c per head
                for h in range(GH):
                    ps = PSM.tile([128, 512], F32, tag="m", name=f"pqc{h}")
                    for lc in range(4):
                        nc.tensor.matmul(
                            ps[0:C, :],
                            lhsT=wqc_sb[:, lc, h * C:(h + 1) * C],
                            rhs=q_t[:, lc, :],
                            start=(lc == 0), stop=(lc == 3),
                        )
                    nc.vector.tensor_copy(qT[0:C, h, tsl], ps[0:C, :])
                    yield
                # q_rope: stacked 4h x 32 rope + rot; combine, scatter
                psr = PSM.tile([128, 512], F32, tag="m", name="pqr")
                psrr = PSM.tile([128, 512], F32, tag="m", name="pqrr")
                for lc in range(4):
                    nc.tensor.matmul(
                        psr[:], lhsT=wqr_sb[:, lc, :],
                        rhs=q_t[:, lc, :],
                        start=(lc == 0), stop=(lc == 3),
                    )
                yield
                for lc in range(4):
                    nc.tensor.matmul(
                        psrr[:], lhsT=wqrr_sb[:, lc, :],
                        rhs=q_t[:, lc, :],
                        start=(lc == 0), stop=(lc == 3),
                    )
                t2b = TMP.tile([128, 512], F32, tag="t2b", name="t2b")
                t3 = TMP.tile([128, 512], BF16, tag="t3b", name="t3b")
                nc.vector.tensor_mul(psr[:], psr[:], cos4[:, tsl])
                nc.vector.tensor_mul(t2b[:], psrr[:], sin4[:, tsl])
                nc.vector.tensor_add(t3[:], psr[:], t2b[:])
                for h in range(GH):
                    nc.sync.dma_start(
                        qT[C:128, h, tsl], t3[32 * h:32 * h + 32, :]
                    )
                yield

            def phC_head(c, h, micro=None, spacing=1, ctr=None):
                """attention main for (chunk c, head h): scores+exp+ctx+tree.
                Returns state for phC_fin.  Diagonal key chunks last so their
                mask-multiply stays off the exp->ctx critical path.  `micro`
                is a generator of small exp-independent PE emissions, consumed
                every `spacing` ctx matmuls to absorb the ACT lag."""
                csl = slice(c * 512, (c + 1) * 512)
                nkc = 4 * (c + 1)
                kcs = list(range(4 * c)) + [4 * c + d for d in range(4)]
                ets = {}
                stack = []  # binary-counter tree: list of (level, tile)

                def emit_score(kc, i):
                    ps = PSS.tile([128, 512], F32, tag="s", name=f"ps{i % 2}")
                    nc.tensor.matmul(
                        ps[:],
                        lhsT=kT[:, h, kc * 128:(kc + 1) * 128],
                        rhs=qT[:, h, csl],
                        start=True, stop=True,
                    )
                    et = ETS.tile([128, 512], BF16, tag="e", name=f"et{i % 6}")
                    nc.scalar.activation(
                        et[:], ps[:], mybir.ActivationFunctionType.Exp
                    )
                    off = kc - 4 * c
                    if off >= 0:
                        nc.vector.tensor_mul(
                            et[:], et[:], mask4[:, off * 512:(off + 1) * 512]
                        )
                    ets[kc] = et

                pctx = PSC.tile([128, 512], F32, tag="c", name="pctx")

                def emit_ctx(kc, i):
                    nc.tensor.matmul(
                        pctx[:],
                        lhsT=vG[:, kc, h * DH:(h + 1) * DH],
                        rhs=ets[kc][:],
                        start=(i == 0), stop=(i == nkc - 1),
                    )
                    # fold into the tree-sum (bf16); level-0 pair adds go to
                    # the otherwise-idle GpSimd, upper levels to DVE
                    carry = ets[kc]
                    lvl = 0
                    while stack and stack[-1][0] == lvl:
                        _, other = stack.pop()
                        dst = ACC.tile([128, 512], BF16, tag="a",
                                       name=f"acc{i % 5}")
                        nc.vector.tensor_add(dst[:], other[:], carry[:])
                        carry = dst
                        lvl += 1
                    stack.append((lvl, carry))

                def tick():
                    if micro is not None and ctr is not None:
                        ctr[0] += 1
                        if ctr[0] % spacing == 0:
                            next(micro, None)

                LAG = 3
                for i, kc in enumerate(kcs):
                    emit_score(kc, i)
                    if i >= LAG:
                        emit_ctx(kcs[i - LAG], i - LAG)
                        tick()
                for i in range(max(0, nkc - LAG), nkc):
                    emit_ctx(kcs[i], i)
                    tick()
                # fold remaining tree levels
                while len(stack) > 1:
                    l1, a = stack.pop()
                    l2, b = stack.pop()
                    dst = ACC.tile([128, 512], BF16, tag="a", name="accf")
                    nc.vector.tensor_add(dst[:], a[:], b[:])
                    stack.append((max(l1, l2) + 1, dst))
                return pctx, stack[0][1]

            def phC_fin(c, h, pctx, tsum):
                """row-sum via all-ones matmul, reciprocal, ctx normalize."""
                csl = slice(c * 512, (c + 1) * 512)
                prs = PSM.tile([128, 512], F32, tag="m", name="prs")
                nc.tensor.matmul(
                    prs[:], lhsT=onesb[:], rhs=tsum[:], start=True, stop=True
                )
                rc = RCP.tile([128, 512], F32, tag="rc", name="rc")
                nc.vector.reciprocal_approx_fast(out=rc[:], in_=prs[:])
                nc.vector.tensor_mul(ctxT[:, h, csl], pctx[:], rc[:])

            def phC(c):
                for h in range(GH):
                    pctx, tsum = phC_head(c, h)
                    phC_fin(c, h, pctx, tsum)

            def load_wo():
                if WO[0] is None:
                    WO[0] = tc.alloc_tile_pool(name="wo", bufs=1, side="right")
                    wo_sb[0] = WO[0].tile([128, 4, HID], BF16, tag="wo",
                                          name="wo")
                    for hc in range(4):
                        nc.sync.dma_start(wo_sb[0][:, hc, :], woT_r[hc])

            def phD_oc(qb, oc):
                """one [128q, 512o] psum group of the partial out-proj."""
                ps = PSM.tile([128, 512], F32, tag="m", name=f"po{oc % 3}")
                for h in range(GH):
                    nc.tensor.matmul(
                        ps[:],
                        lhsT=ctxT[:, h, qb * 128:(qb + 1) * 128],
                        rhs=wo_sb[0][:, h, oc * 512:(oc + 1) * 512],
                        start=(h == 0), stop=(h == 3),
                    )
                ot = OT.tile([128, 512], F32, tag="ot", name="ot")
                nc.vector.tensor_copy(ot[:], ps[:])
                nc.sync.dma_start(
                    out_d[qb * 128:(qb + 1) * 128, oc * 512:(oc + 1) * 512],
                    ot[:],
                )

            def phD_qb(qb):
                """partial out-projection for one 128-row query block."""
                for oc in range(4):
                    phD_oc(qb, oc)

            def phD_gen(qbs):
                for qb in qbs:
                    for oc in range(4):
                        phD_oc(qb, oc)
                        yield

            # ---------------- master schedule ----------------
            xt0 = load_x(0)
            load_wd()
            load_cossin()
            # warm-up matmuls on a memset tile: PE starts at ~+2us (no DMA
            # dependency), so HAM is un-throttled before the real work
            wmt = perB.tile([128, 512], BF16, tag="wmt", name="wmt")
            nc.gpsimd.memset(wmt[:], 0.0)
            for i in range(26):
                pw = PSM.tile([128, 512], F32, tag="m", name=f"warm{i % 2}")
                nc.tensor.matmul(pw[:], lhsT=wmt[:, 0:128], rhs=wmt[:],
                                 start=True, stop=True)
            from itertools import chain as _chain

            lat0 = phA(0, xt0)
            load_aux_weights()
            for _ in phB_gen(0, *lat0):
                pass
            lat1 = phA(1)

            def run_chunk(c, micro, spacing):
                ctr = [0]
                for h in range(GH):
                    st = phC_head(c, h, micro, spacing, ctr)
                    phC_fin(c, h, *st)
                for _ in micro:   # drain leftover pieces
                    pass

            # micro-fillers keep the PE fed while ACT does the exps
            run_chunk(0, phB_gen(1, *lat1), 1)
            lat2 = phA(2)
            run_chunk(1, phB_gen(2, *lat2), 2)
            lat3 = phA(3)
            WD.release()
            load_wo()
            run_chunk(2, _chain(phB_gen(3, *lat3), phD_gen([0])), 2)
            run_chunk(3, phD_gen(range(1, 9)), 2)
            for qb in range(9, 16):
                phD_qb(qb)
            if WO[0] is not None:
                WO[0].release()

    nc.compile()
    return nc


def _rot_rows(w):
    # rows of w are the rope dim; rot(w) @ lat == rotate_half(w @ lat)
    hR = w.shape[0] // 2
    return np.concatenate([-w[hR:], w[:hR]], axis=0)


def _prep_inputs(inputs):
    x = np.asarray(inputs["x"], np.float32)
    Wq_down = np.asarray(inputs["Wq_down"], np.float32)
    Wq_up = np.asarray(inputs["Wq_up"], np.float32)
    Wq_rope = np.asarray(inputs["Wq_rope"], np.float32)
    Wkv_down = np.asarray(inputs["Wkv_down"], np.float32)
    Wk_up = np.asarray(inputs["Wk_up"], np.float32)
    Wk_rope = np.asarray(inputs["Wk_rope"], np.float32)
    Wv_up = np.asarray(inputs["Wv_up"], np.float32)
    Wo = np.asarray(inputs["Wo"], np.float32)

    s = np.float32(1.0 / np.sqrt(DH))

    wd_kvT = np.ascontiguousarray(Wkv_down.T).astype(BF16NP)
    wd_qT = np.ascontiguousarray(Wq_down.T).astype(BF16NP)
    wkr2 = np.concatenate([Wk_rope, _rot_rows(Wk_rope)], axis=0)  # [64, HID]
    wkr2T = np.ascontiguousarray(wkr2.T).astype(BF16NP)

    inv_freq = (1.0 / (10000.0 ** (np.arange(0, R, 2, dtype=np.float32) / R)))
    t = np.arange(S, dtype=np.float32)
    freqs = t[:, None] * inv_freq[None, :]
    emb = np.concatenate([freqs, freqs], axis=-1)          # [S, R]
    cos4 = np.tile(np.cos(emb).T, (4, 1)).astype(BF16NP)   # [128, S]
    sin4 = np.tile(np.sin(emb).T, (4, 1)).astype(BF16NP)

    kar = np.arange(128)[:, None]
    qar = np.arange(512)[None, :]
    mask4 = np.empty((128, NQC * 512), np.float32)
    for off in range(4):
        mask4[:, off * 512:(off + 1) * 512] = (
            (128 * off + kar) <= qar
        ).astype(np.float32)
    mask4 = mask4.astype(BF16NP)

    per_g = []
    for g in range(4):
        hsl = slice(g * GH, (g + 1) * GH)
        wk_p = np.concatenate(
            [Wk_up[h * C:(h + 1) * C] for h in range(g * GH, (g + 1) * GH)],
            axis=0)                                         # [384, LAT]
        wv_p = np.concatenate(
            [Wv_up[h * DH:(h + 1) * DH] for h in range(g * GH, (g + 1) * GH)],
            axis=0)                                         # [512, LAT]
        wqc_p = np.concatenate(
            [Wq_up[h * C:(h + 1) * C] for h in range(g * GH, (g + 1) * GH)],
            axis=0) * s
        wqr_p = np.concatenate(
            [Wq_rope[h * R:(h + 1) * R] for h in range(g * GH, (g + 1) * GH)],
            axis=0) * s
        wqrr_p = np.concatenate(
            [_rot_rows(Wq_rope[h * R:(h + 1) * R])
             for h in range(g * GH, (g + 1) * GH)], axis=0) * s
        wo_g = Wo[:, g * GH * DH:(g + 1) * GH * DH]         # [HID, 512]
        per_g.append({
            "wk_pT": np.ascontiguousarray(wk_p.T).astype(BF16NP),
            "wv_pT": np.ascontiguousarray(wv_p.T).astype(BF16NP),
            "wqc_pT": np.ascontiguousarray(wqc_p.T).astype(BF16NP),
            "wqr_pT": np.ascontiguousarray(wqr_p.T).astype(BF16NP),
            "wqrr_pT": np.ascontiguousarray(wqrr_p.T).astype(BF16NP),
            "woT": np.ascontiguousarray(wo_g.T).astype(BF16NP),
        })

    in_maps = []
    for cid in range(8):
        b, g = divmod(cid, 4)
        m = {
            "xbT": np.ascontiguousarray(x[b].T).astype(BF16NP),
            "wd_kvT": wd_kvT, "wd_qT": wd_qT, "wkr2T": wkr2T,
            "cos4": cos4, "sin4": sin4, "mask4": mask4,
        }
        m.update(per_g[g])
        in_maps.append(m)
    return in_maps


_NC_CACHE = None


def kernel(**inputs):
    global _NC_CACHE
    if _NC_CACHE is None:
        _NC_CACHE = build_nc()
    nc = _NC_CACHE
    in_maps = _prep_inputs(inputs)
    res = run_bass_kernel_spmd(nc, in_maps, list(range(8)))
    bo = np.asarray(inputs["bo"], np.float32)
    out = np.empty((B, S, HID), np.float32)
    for b in range(B):
        acc = res.results[4 * b]["out"].astype(np.float32)
        for g in range(1, 4):
            acc = acc + res.results[4 * b + g]["out"]
        out[b] = acc + bo
    return out


# revision 60
# speedup vs baseline: 1.0914x; 1.0164x over previous
"""MLA (multi-head latent attention) Trainium2 kernel, SPMD over 8 NeuronCores.

Sharding: core c = 4*b + g handles batch b and head group g (4 heads),
ALL 2048 query rows.  Causality: query chunk c (512 rows) only attends
key chunks 0..4c+3 (lower triangle), so every core does the same
triangular work -- perfectly balanced, no masks off the diagonal.
Each core emits a PARTIAL out-projection (contraction over its 4 heads'
128-dims); the host sums the 4 partials per batch (+bias).  No
collectives.

On-chip layouts are transposed ([feature, token]) so every matmul
contracts over the partition dim with no on-chip transposes.
rotate_half is folded into host-permuted weight copies; 1/sqrt(dh) into
the q weights; softmax skips the max-pass (scores bounded) and gets its
row-sum from an all-ones matmul over a DVE tree-sum of the exp tiles.
Diagonal score tiles are masked multiplicatively (0/1 bf16) after exp.
"""

import os
import sys
import types

for _p in ("/opt/trn_rl_repo", "/root/.axon_site/_ro/trn_rl_repo"):
    if os.path.isdir(_p) and _p not in sys.path:
        sys.path.append(_p)

import numpy as np
import ml_dtypes

import concourse.bass as bass
import concourse.bacc as bacc_mod
import concourse.mybir as mybir
from concourse.tile import TileContext
from concourse.vector_clock import ScopedClock
from concourse.bass_utils import run_bass_kernel_spmd

F32 = mybir.dt.float32
BF16 = mybir.dt.bfloat16
BF16NP = ml_dtypes.bfloat16

HID, H, LAT, R, DH, C = 2048, 16, 512, 32, 128, 96
B, S = 2, 2048
GH = 4            # heads per core
NQC = 4           # query chunks of 512
NKC = 16          # key chunks of 128


def _patch_tile_drain():
    """The staged walrus rejects a Drain carrying >1 sync-wait. Move the
    TileContext tail-drain waits onto single-wait SP nops."""

    def _drain_and_barrier(self, tick_clock, wait_clock):
        drain_inst = self.nc.sync.drain()
        wait_clock.add_sem_waits(
            drain_inst.ins, ScopedClock({None: tick_clock.global_clock})
        )
        si = drain_inst.ins.sync_info
        if si is not None and len(si.on_wait) > 1:
            waits = list(si.on_wait)
            drain_inst.ins.sync_info = mybir.SyncInfo(
                on_wait=[], on_update=list(si.on_update)
            )
            for w in waits:
                nop = self.nc.sync.nop(nofuse=True)
                nop.ins.sync_info = mybir.SyncInfo(on_wait=[w], on_update=[])
        self.nc.all_engine_barrier()
        assert self.sems is not None
        popped = self.nc._tile_sem_poison_stack.pop()
        assert popped is self._sem_poison
        self.nc.clear_and_free_semaphores(list(self.sems.allocated().values()))
        self.nc.all_engine_barrier()

    TileContext._drain_and_barrier = _drain_and_barrier


def _install_ntff_hook():
    """antenv.axon_hooks is absent in this image; inject it and register the
    ctypes NTFF hook so trace=True / BASS_TRACE can profile."""
    try:
        import antenv

        if "antenv.axon_hooks" not in sys.modules:
            mod = types.ModuleType("antenv.axon_hooks")
            mod._hook = None

            def set_axon_ntff_profile_hook(h):
                mod._hook = h

            def get_axon_ntff_profile_hook():
                return mod._hook

            mod.set_axon_ntff_profile_hook = set_axon_ntff_profile_hook
            mod.get_axon_ntff_profile_hook = get_axon_ntff_profile_hook
            sys.modules["antenv.axon_hooks"] = mod
            antenv.axon_hooks = mod
        boot_dir = "/root/.axon_site/trn_agent_boot"
        so_path = "/opt/axon/libaxon_pjrt.so"
        if os.path.isdir(boot_dir) and os.path.exists(so_path):
            if boot_dir not in sys.path:
                sys.path.append(boot_dir)
            from trn_boot import _ntff_profile_via_ctypes

            hook = _ntff_profile_via_ctypes(so_path)
            if hook is not None:
                sys.modules["antenv.axon_hooks"].set_axon_ntff_profile_hook(hook)
    except Exception:
        pass


_patch_tile_drain()
_install_ntff_hook()


def _dram(nc, name, shape, dtype=F32, out=False):
    return nc.declare_dram_parameter(name, list(shape), dtype, isOutput=out)


def build_nc():
    nc = bacc_mod.Bacc("TRN2")

    xbT = _dram(nc, "xbT", [HID, S], BF16)            # x[b].T
    wd_kvT = _dram(nc, "wd_kvT", [HID, LAT], BF16)    # Wkv_down.T
    wd_qT = _dram(nc, "wd_qT", [HID, LAT], BF16)      # Wq_down.T
    wkr2T = _dram(nc, "wkr2T", [HID, 2 * R], BF16)    # [Wk_rope; rot].T
    wk_pT = _dram(nc, "wk_pT", [LAT, GH * C], BF16)   # 4-head k_c pack .T
    wv_pT = _dram(nc, "wv_pT", [LAT, GH * DH], BF16)  # 4-head v pack .T
    wqc_pT = _dram(nc, "wqc_pT", [LAT, GH * C], BF16)   # 4-head q_c pack /sqrt
    wqr_pT = _dram(nc, "wqr_pT", [LAT, GH * R], BF16)   # 4-head q_rope /sqrt
    wqrr_pT = _dram(nc, "wqrr_pT", [LAT, GH * R], BF16)  # rotated rope /sqrt
    woT = _dram(nc, "woT", [GH * DH, HID], BF16)      # Wo cols for our heads
    cos4_d = _dram(nc, "cos4", [128, S], BF16)        # cos.T tiled 4x
    sin4_d = _dram(nc, "sin4", [128, S], BF16)
    mask4_d = _dram(nc, "mask4", [128, NQC * 512], BF16)  # 0/1 diag masks
    out_d = _dram(nc, "out", [S, HID], out=True)      # partial (4-head) proj

    xbT_r = xbT[:, :].rearrange("(c p two) t -> c p two t", p=128, two=2)
    wd_kvT_r = wd_kvT[:, :].rearrange("(c p two) l -> c p two l", p=128, two=2)
    wd_qT_r = wd_qT[:, :].rearrange("(c p two) l -> c p two l", p=128, two=2)
    wkr2T_r = wkr2T[:, :].rearrange("(c p two) r -> c p two r", p=128, two=2)
    wk_pT_r = wk_pT[:, :].rearrange("(lc p) d -> lc p d", p=128)
    wv_pT_r = wv_pT[:, :].rearrange("(lc p) d -> lc p d", p=128)
    wqc_pT_r = wqc_pT[:, :].rearrange("(lc p) d -> lc p d", p=128)
    wqr_pT_r = wqr_pT[:, :].rearrange("(lc p) d -> lc p d", p=128)
    wqrr_pT_r = wqrr_pT[:, :].rearrange("(lc p) d -> lc p d", p=128)
    woT_r = woT[:, :].rearrange("(hc p) o -> hc p o", p=128)

    with TileContext(nc) as tc:
        with tc.tile_pool(name="perB", bufs=1) as perB, \
             tc.tile_pool(name="lat", bufs=2) as LATP, \
             tc.tile_pool(name="xs", bufs=1) as XS, \
             tc.tile_pool(name="ets", bufs=6) as ETS, \
             tc.tile_pool(name="acc", bufs=8) as ACC, \
             tc.tile_pool(name="rcp", bufs=2) as RCP, \
             tc.tile_pool(name="tmp", bufs=2) as TMP, \
             tc.tile_pool(name="ot", bufs=3) as OT, \
             tc.tile_pool(name="ps_g", bufs=2, space="PSUM") as PSG, \
             tc.tile_pool(name="ps_m", bufs=2, space="PSUM") as PSM, \
             tc.tile_pool(name="ps_s", bufs=2, space="PSUM") as PSS, \
             tc.tile_pool(name="ps_c", bufs=2, space="PSUM") as PSC:

            # ---------- persistent SBUF ----------
            krT = perB.tile([32, S], BF16, tag="krT", name="krT")
            kT = perB.tile([128, GH, S], BF16, tag="kT", name="kT")
            vG = perB.tile([128, NKC, GH * DH], BF16, tag="vG", name="vG")
            qT = perB.tile([128, GH, S], BF16, tag="qT", name="qT")
            ctxT = perB.tile([128, GH, S], BF16, tag="ctxT", name="ctxT")
            cos4 = perB.tile([128, S], BF16, tag="cos4", name="cos4")
            sin4 = perB.tile([128, S], BF16, tag="sin4", name="sin4")
            mask4 = perB.tile([128, NQC * 512], BF16, tag="mask4", name="mask4")
            onesb = perB.tile([128, 128], BF16, tag="ones", name="ones")
            wk_sb = perB.tile([128, 4, GH * C], BF16, tag="wk", name="wk")
            wv_sb = perB.tile([128, 4, GH * DH], BF16, tag="wv", name="wv")
            wqc_sb = perB.tile([128, 4, GH * C], BF16, tag="wqc", name="wqc")
            wqr_sb = perB.tile([128, 4, GH * R], BF16, tag="wqr", name="wqr")
            wqrr_sb = perB.tile([128, 4, GH * R], BF16, tag="wqrr", name="wqrr")

            # down-proj weights: released after phase A(3), wo loaded after.
            # Per-hc tiles so the first matmuls wait only on their own slice;
            # kv weights first (the very first accumulation pass).
            WD = tc.alloc_tile_pool(name="wd", bufs=1, side="right")
            wdkv = [WD.tile([128, 2, LAT], BF16, tag=f"wdkv{hc}",
                            name=f"wdkv{hc}") for hc in range(8)]
            wdq = [WD.tile([128, 2, LAT], BF16, tag=f"wdq{hc}",
                           name=f"wdq{hc}") for hc in range(8)]
            wkr = [WD.tile([128, 2, 2 * R], BF16, tag=f"wkr{hc}",
                           name=f"wkr{hc}") for hc in range(8)]

            def load_wd():
                for hc in range(8):
                    nc.sync.dma_start(wdkv[hc][:], wd_kvT_r[hc])
                for hc in range(8):
                    nc.sync.dma_start(wkr[hc][:], wkr2T_r[hc])
                    nc.sync.dma_start(wdq[hc][:], wd_qT_r[hc])

            def load_x(tq):
                tsl = slice(tq * 512, (tq + 1) * 512)
                xt = [XS.tile([128, 2, 512], BF16, tag=f"xf{hc}",
                              name=f"xf{hc}") for hc in range(8)]
                for hc in range(8):
                    nc.sync.dma_start(xt[hc][:], xbT_r[hc][:, :, tsl])
                return xt

            WO = [None]  # box for the late wo pool
            wo_sb = [None]

            def load_cossin():
                nc.sync.dma_start(cos4[:], cos4_d[:, :])
                nc.sync.dma_start(sin4[:], sin4_d[:, :])

            def load_aux_weights():
                nc.sync.dma_start(mask4[:], mask4_d[:, :])
                nc.gpsimd.memset(onesb[:], 1.0)
                for lc in range(4):
                    nc.sync.dma_start(wk_sb[:, lc, :], wk_pT_r[lc])
                    nc.sync.dma_start(wv_sb[:, lc, :], wv_pT_r[lc])
                    nc.sync.dma_start(wqc_sb[:, lc, :], wqc_pT_r[lc])
                    nc.sync.dma_start(wqr_sb[:, lc, :], wqr_pT_r[lc])
                    nc.sync.dma_start(wqrr_sb[:, lc, :], wqrr_pT_r[lc])

            # ---------------- phase emitters ----------------
            def phA(tq, xt=None):
                """latents for token quarter tq: kv_lat, roped k_rope, q_lat.
                Returns the per-quarter latent tiles for phB(tq)."""
                tsl = slice(tq * 512, (tq + 1) * 512)
                if xt is None:
                    xt = load_x(tq)
                kv_t = LATP.tile([128, 4, 512], BF16, tag="kvlat",
                                 name="kvlat")
                q_t = LATP.tile([128, 4, 512], BF16, tag="qlat", name="qlat")

                # kv_lat: 4 lc passes, 2 rotating psum banks
                for lc in range(4):
                    ps = PSG.tile([128, 512], F32, tag="g", name=f"pkv{lc}")
                    for hc in range(8):
                        for two in range(2):
                            nc.tensor.matmul(
                                ps[:],
                                lhsT=wdkv[hc][:, two, lc * 128:(lc + 1) * 128],
                                rhs=xt[hc][:, two, :],
                                start=(hc == 0 and two == 0),
                                stop=(hc == 7 and two == 1),
                            )
                    nc.vector.tensor_copy(kv_t[:, lc, :], ps[:])
                # k_rope pass (64 rows: [rope; rot]); combine in place
                pkr = PSG.tile([64, 512], F32, tag="g", name="pkr")
                for hc in range(8):
                    for two in range(2):
                        nc.tensor.matmul(
                            pkr[:],
                            lhsT=wkr[hc][:, two, :],
                            rhs=xt[hc][:, two, :],
                            start=(hc == 0 and two == 0),
                            stop=(hc == 7 and two == 1),
                        )
                nc.vector.tensor_mul(pkr[0:32, :], pkr[0:32, :],
                                     cos4[0:32, tsl])
                tkr = TMP.tile([32, 512], F32, tag="tkr", name="tkr")
                nc.vector.tensor_mul(tkr[:], pkr[32:64, :], sin4[0:32, tsl])
                nc.vector.tensor_add(krT[:, tsl], pkr[0:32, :], tkr[:])
                # q_lat: 4 lc passes
                for lc in range(4):
                    ps = PSG.tile([128, 512], F32, tag="g", name=f"pq{lc}")
                    for hc in range(8):
                        for two in range(2):
                            nc.tensor.matmul(
                                ps[:],
                                lhsT=wdq[hc][:, two, lc * 128:(lc + 1) * 128],
                                rhs=xt[hc][:, two, :],
                                start=(hc == 0 and two == 0),
                                stop=(hc == 7 and two == 1),
                            )
                    nc.vector.tensor_copy(q_t[:, lc, :], ps[:])
                return kv_t, q_t

            def phB_gen(tq, kv_t, q_t):
                """per-head projections for quarter tq, as a generator of
                ~0.9us PE pieces (one psum group each)."""
                tsl = slice(tq * 512, (tq + 1) * 512)
                # k_c per head (96 content rows)
                for h in range(GH):
                    ps = PSM.tile([128, 512], F32, tag="m", name=f"pk{h}")
                    for lc in range(4):
                        nc.tensor.matmul(
                            ps[0:C, :],
                            lhsT=wk_sb[:, lc, h * C:(h + 1) * C],
                            rhs=kv_t[:, lc, :],
                            start=(lc == 0), stop=(lc == 3),
                        )
                    nc.vector.tensor_copy(kT[0:C, h, tsl], ps[0:C, :])
                    nc.sync.dma_start(kT[C:128, h, tsl], krT[:, tsl])
                    yield
                # v: 4 token sub-chunks of 128, out = [t, 4h*128]
                for t2 in range(4):
                    kc = tq * 4 + t2
                    ps = PSM.tile([128, 512], F32, tag="m", name=f"pv{t2}")
                    for lc in range(4):
                        nc.tensor.matmul(
                            ps[:],
                            lhsT=kv_t[:, lc, t2 * 128:(t2 + 1) * 128],
                            rhs=wv_sb[:, lc, :],
                            start=(lc == 0), stop=(lc == 3),
                        )
                    nc.vector.tensor_copy(vG[:, kc, :], ps[:])
                    yield
                # q_c per head
                for h in range(GH):
                    ps = PSM.tile([128, 512], F32, tag="m", name=f"pqc{h}")
                    for lc in range(4):
                        nc.tensor.matmul(
                            ps[0:C, :],
                            lhsT=wqc_sb[:, lc, h * C:(h + 1) * C],
                            rhs=q_t[:, lc, :],
                            start=(lc == 0), stop=(lc == 3),
                        )
                    nc.vector.tensor_copy(qT[0:C, h, tsl], ps[0:C, :])
                    yield
                # q_rope: stacked 4h x 32 rope + rot; combine, scatter
                psr = PSM.tile([128, 512], F32, tag="m", name="pqr")
                psrr = PSM.tile([128, 512], F32, tag="m", name="pqrr")
                for lc in range(4):
                    nc.tensor.matmul(
                        psr[:], lhsT=wqr_sb[:, lc, :],
                        rhs=q_t[:, lc, :],
                        start=(lc == 0), stop=(lc == 3),
                    )
                yield
                for lc in range(4):
                    nc.tensor.matmul(
                        psrr[:], lhsT=wqrr_sb[:, lc, :],
                        rhs=q_t[:, lc, :],
                        start=(lc == 0), stop=(lc == 3),
                    )
                t2b = TMP.tile([128, 512], F32, tag="t2b", name="t2b")
                t3 = TMP.tile([128, 512], BF16, tag="t3b", name="t3b")
                nc.vector.tensor_mul(psr[:], psr[:], cos4[:, tsl])
                nc.vector.tensor_mul(t2b[:], psrr[:], sin4[:, tsl])
                nc.vector.tensor_add(t3[:], psr[:], t2b[:])
                for h in range(GH):
                    nc.sync.dma_start(
                        qT[C:128, h, tsl], t3[32 * h:32 * h + 32, :]
                    )
                yield

            def phC_head(c, h, micro=None, spacing=1, ctr=None):
                """attention main for (chunk c, head h): scores+exp+ctx+tree.
                Returns state for phC_fin.  Diagonal key chunks last so their
                mask-multiply stays off the exp->ctx critical path.  `micro`
                is a generator of small exp-independent PE emissions, consumed
                every `spacing` ctx matmuls to absorb the ACT lag."""
                csl = slice(c * 512, (c + 1) * 512)
                nkc = 4 * (c + 1)
                kcs = list(range(4 * c)) + [4 * c + d for d in range(4)]
                ets = {}
                stack = []  # binary-counter tree: list of (level, tile)

                def emit_score(kc, i):
                    ps = PSS.tile([128, 512], F32, tag="s", name=f"ps{i % 2}")
                    nc.tensor.matmul(
                        ps[:],
                        lhsT=kT[:, h, kc * 128:(kc + 1) * 128],
                        rhs=qT[:, h, csl],
                        start=True, stop=True,
                    )
                    et = ETS.tile([128, 512], BF16, tag="e", name=f"et{i % 6}")
                    nc.scalar.activation(
                        et[:], ps[:], mybir.ActivationFunctionType.Exp
                    )
                    off = kc - 4 * c
                    if off >= 0:
                        nc.vector.tensor_mul(
                            et[:], et[:], mask4[:, off * 512:(off + 1) * 512]
                        )
                    ets[kc] = et

                pctx = PSC.tile([128, 512], F32, tag="c", name="pctx")

                def emit_ctx(kc, i):
                    nc.tensor.matmul(
                        pctx[:],
                        lhsT=vG[:, kc, h * DH:(h + 1) * DH],
                        rhs=ets[kc][:],
                        start=(i == 0), stop=(i == nkc - 1),
                    )
                    # fold into the tree-sum (bf16); level-0 pair adds go to
                    # the otherwise-idle GpSimd, upper levels to DVE
                    carry = ets[kc]
                    lvl = 0
                    while stack and stack[-1][0] == lvl:
                        _, other = stack.pop()
                        dst = ACC.tile([128, 512], BF16, tag="a",
                                       name=f"acc{i % 5}")
                        nc.vector.tensor_add(dst[:], other[:], carry[:])
                        carry = dst
                        lvl += 1
                    stack.append((lvl, carry))

                def tick():
                    if micro is not None and ctr is not None:
                        ctr[0] += 1
                        if ctr[0] % spacing == 0:
                            next(micro, None)

                LAG = 3
                for i, kc in enumerate(kcs):
                    emit_score(kc, i)
                    if i >= LAG:
                        emit_ctx(kcs[i - LAG], i - LAG)
                        tick()
                for i in range(max(0, nkc - LAG), nkc):
                    emit_ctx(kcs[i], i)
                    tick()
                # fold remaining tree levels
                while len(stack) > 1:
                    l1, a = stack.pop()
                    l2, b = stack.pop()
                    dst = ACC.tile([128, 512], BF16, tag="a", name="accf")
                    nc.vector.tensor_add(dst[:], a[:], b[:])
                    stack.append((max(l1, l2) + 1, dst))
                return pctx, stack[0][1]

            def phC_fin(c, h, pctx, tsum):
                """row-sum via all-ones matmul, reciprocal, ctx normalize."""
                csl = slice(c * 512, (c + 1) * 512)
                prs = PSM.tile([128, 512], F32, tag="m", name="prs")
                nc.tensor.matmul(
                    prs[:], lhsT=onesb[:], rhs=tsum[:], start=True, stop=True
                )
                rc = RCP.tile([128, 512], F32, tag="rc", name="rc")
                nc.vector.reciprocal_approx_fast(out=rc[:], in_=prs[:])
                nc.vector.tensor_mul(ctxT[:, h, csl], pctx[:], rc[:])

            def phC(c):
                for h in range(GH):
                    pctx, tsum = phC_head(c, h)
                    phC_fin(c, h, pctx, tsum)

            def load_wo():
                if WO[0] is None:
                    WO[0] = tc.alloc_tile_pool(name="wo", bufs=1, side="right")
                    wo_sb[0] = WO[0].tile([128, 4, HID], BF16, tag="wo",
                                          name="wo")
                    for hc in range(4):
                        nc.sync.dma_start(wo_sb[0][:, hc, :], woT_r[hc])

            def phD_oc(qb, oc, on_act=False):
                """one [128q, 512o] psum group of the partial out-proj.
                phD runs only after phA(3), so it owns the PSG banks; in the
                exp-free tail the evac copy goes on the idle ACT engine."""
                ps = PSG.tile([128, 512], F32, tag="g", name=f"po{oc % 3}")
                for h in range(GH):
                    nc.tensor.matmul(
                        ps[:],
                        lhsT=ctxT[:, h, qb * 128:(qb + 1) * 128],
                        rhs=wo_sb[0][:, h, oc * 512:(oc + 1) * 512],
                        start=(h == 0), stop=(h == 3),
                    )
                ot = OT.tile([128, 512], F32, tag="ot", name="ot")
                if on_act:
                    nc.scalar.copy(ot[:], ps[:])
                else:
                    nc.vector.tensor_copy(ot[:], ps[:])
                nc.sync.dma_start(
                    out_d[qb * 128:(qb + 1) * 128, oc * 512:(oc + 1) * 512],
                    ot[:],
                )

            def phD_qb(qb, on_act=False):
                """partial out-projection for one 128-row query block."""
                for oc in range(4):
                    phD_oc(qb, oc, on_act)

            def phD_gen(qbs):
                for qb in qbs:
                    for oc in range(4):
                        phD_oc(qb, oc)
                        yield

            # ---------------- master schedule ----------------
            xt0 = load_x(0)
            load_wd()
            load_cossin()
            # warm-up matmuls on a memset tile: PE starts at ~+2us (no DMA
            # dependency), so HAM is un-throttled before the real work
            wmt = perB.tile([128, 512], BF16, tag="wmt", name="wmt")
            nc.gpsimd.memset(wmt[:], 0.0)
            for i in range(26):
                pw = PSM.tile([128, 512], F32, tag="m", name=f"warm{i % 2}")
                nc.tensor.matmul(pw[:], lhsT=wmt[:, 0:128], rhs=wmt[:],
                                 start=True, stop=True)
            from itertools import chain as _chain

            lat0 = phA(0, xt0)
            load_aux_weights()
            for _ in phB_gen(0, *lat0):
                pass
            lat1 = phA(1)

            def run_chunk(c, micro, spacing):
                ctr = [0]
                for h in range(GH):
                    st = phC_head(c, h, micro, spacing, ctr)
                    phC_fin(c, h, *st)
                for _ in micro:   # drain leftover pieces
                    pass

            # micro-fillers keep the PE fed while ACT does the exps
            run_chunk(0, phB_gen(1, *lat1), 1)
            lat2 = phA(2)
            run_chunk(1, phB_gen(2, *lat2), 2)
            lat3 = phA(3)
            WD.release()
            load_wo()
            run_chunk(2, _chain(phB_gen(3, *lat3), phD_gen([0])), 2)
            run_chunk(3, phD_gen(range(1, 9)), 2)
            for qb in range(9, 16):
                phD_qb(qb, on_act=(qb % 2 == 1))
            if WO[0] is not None:
                WO[0].release()

    nc.compile()
    return nc


def _rot_rows(w):
    # rows of w are the rope dim; rot(w) @ lat == rotate_half(w @ lat)
    hR = w.shape[0] // 2
    return np.concatenate([-w[hR:], w[:hR]], axis=0)


def _prep_inputs(inputs):
    x = np.asarray(inputs["x"], np.float32)
    Wq_down = np.asarray(inputs["Wq_down"], np.float32)
    Wq_up = np.asarray(inputs["Wq_up"], np.float32)
    Wq_rope = np.asarray(inputs["Wq_rope"], np.float32)
    Wkv_down = np.asarray(inputs["Wkv_down"], np.float32)
    Wk_up = np.asarray(inputs["Wk_up"], np.float32)
    Wk_rope = np.asarray(inputs["Wk_rope"], np.float32)
    Wv_up = np.asarray(inputs["Wv_up"], np.float32)
    Wo = np.asarray(inputs["Wo"], np.float32)

    s = np.float32(1.0 / np.sqrt(DH))

    wd_kvT = np.ascontiguousarray(Wkv_down.T).astype(BF16NP)
    wd_qT = np.ascontiguousarray(Wq_down.T).astype(BF16NP)
    wkr2 = np.concatenate([Wk_rope, _rot_rows(Wk_rope)], axis=0)  # [64, HID]
    wkr2T = np.ascontiguousarray(wkr2.T).astype(BF16NP)

    inv_freq = (1.0 / (10000.0 ** (np.arange(0, R, 2, dtype=np.float32) / R)))
    t = np.arange(S, dtype=np.float32)
    freqs = t[:, None] * inv_freq[None, :]
    emb = np.concatenate([freqs, freqs], axis=-1)          # [S, R]
    cos4 = np.tile(np.cos(emb).T, (4, 1)).astype(BF16NP)   # [128, S]
    sin4 = np.tile(np.sin(emb).T, (4, 1)).astype(BF16NP)

    kar = np.arange(128)[:, None]
    qar = np.arange(512)[None, :]
    mask4 = np.empty((128, NQC * 512), np.float32)
    for off in range(4):
        mask4[:, off * 512:(off + 1) * 512] = (
            (128 * off + kar) <= qar
        ).astype(np.float32)
    mask4 = mask4.astype(BF16NP)

    per_g = []
    for g in range(4):
        hsl = slice(g * GH, (g + 1) * GH)
        wk_p = np.concatenate(
            [Wk_up[h * C:(h + 1) * C] for h in range(g * GH, (g + 1) * GH)],
            axis=0)                                         # [384, LAT]
        wv_p = np.concatenate(
            [Wv_up[h * DH:(h + 1) * DH] for h in range(g * GH, (g + 1) * GH)],
            axis=0)                                         # [512, LAT]
        wqc_p = np.concatenate(
            [Wq_up[h * C:(h + 1) * C] for h in range(g * GH, (g + 1) * GH)],
            axis=0) * s
        wqr_p = np.concatenate(
            [Wq_rope[h * R:(h + 1) * R] for h in range(g * GH, (g + 1) * GH)],
            axis=0) * s
        wqrr_p = np.concatenate(
            [_rot_rows(Wq_rope[h * R:(h + 1) * R])
             for h in range(g * GH, (g + 1) * GH)], axis=0) * s
        wo_g = Wo[:, g * GH * DH:(g + 1) * GH * DH]         # [HID, 512]
        per_g.append({
            "wk_pT": np.ascontiguousarray(wk_p.T).astype(BF16NP),
            "wv_pT": np.ascontiguousarray(wv_p.T).astype(BF16NP),
            "wqc_pT": np.ascontiguousarray(wqc_p.T).astype(BF16NP),
            "wqr_pT": np.ascontiguousarray(wqr_p.T).astype(BF16NP),
            "wqrr_pT": np.ascontiguousarray(wqrr_p.T).astype(BF16NP),
            "woT": np.ascontiguousarray(wo_g.T).astype(BF16NP),
        })

    in_maps = []
    for cid in range(8):
        b, g = divmod(cid, 4)
        m = {
            "xbT": np.ascontiguousarray(x[b].T).astype(BF16NP),
            "wd_kvT": wd_kvT, "wd_qT": wd_qT, "wkr2T": wkr2T,
            "cos4": cos4, "sin4": sin4, "mask4": mask4,
        }
        m.update(per_g[g])
        in_maps.append(m)
    return in_maps


_NC_CACHE = None


def kernel(**inputs):
    global _NC_CACHE
    if _NC_CACHE is None:
        _NC_CACHE = build_nc()
    nc = _NC_CACHE
    in_maps = _prep_inputs(inputs)
    res = run_bass_kernel_spmd(nc, in_maps, list(range(8)))
    bo = np.asarray(inputs["bo"], np.float32)
    out = np.empty((B, S, HID), np.float32)
    for b in range(B):
        acc = res.results[4 * b]["out"].astype(np.float32)
        for g in range(1, 4):
            acc = acc + res.results[4 * b + g]["out"]
        out[b] = acc + bo
    return out


# revision 61
# speedup vs baseline: 1.1038x; 1.0114x over previous
"""MLA (multi-head latent attention) Trainium2 kernel, SPMD over 8 NeuronCores.

Sharding: core c = 4*b + g handles batch b and head group g (4 heads),
ALL 2048 query rows.  Causality: query chunk c (512 rows) only attends
key chunks 0..4c+3 (lower triangle), so every core does the same
triangular work -- perfectly balanced, no masks off the diagonal.
Each core emits a PARTIAL out-projection (contraction over its 4 heads'
128-dims); the host sums the 4 partials per batch (+bias).  No
collectives.

On-chip layouts are transposed ([feature, token]) so every matmul
contracts over the partition dim with no on-chip transposes.
rotate_half is folded into host-permuted weight copies; 1/sqrt(dh) into
the q weights; softmax skips the max-pass (scores bounded) and gets its
row-sum from an all-ones matmul over a DVE tree-sum of the exp tiles.
Diagonal score tiles are masked multiplicatively (0/1 bf16) after exp.
"""

import os
import sys
import types

for _p in ("/opt/trn_rl_repo", "/root/.axon_site/_ro/trn_rl_repo"):
    if os.path.isdir(_p) and _p not in sys.path:
        sys.path.append(_p)

import numpy as np
import ml_dtypes

import concourse.bass as bass
import concourse.bacc as bacc_mod
import concourse.mybir as mybir
from concourse.tile import TileContext
from concourse.vector_clock import ScopedClock
from concourse.bass_utils import run_bass_kernel_spmd

F32 = mybir.dt.float32
BF16 = mybir.dt.bfloat16
BF16NP = ml_dtypes.bfloat16

HID, H, LAT, R, DH, C = 2048, 16, 512, 32, 128, 96
B, S = 2, 2048
GH = 4            # heads per core
NQC = 4           # query chunks of 512
NKC = 16          # key chunks of 128


def _patch_tile_drain():
    """The staged walrus rejects a Drain carrying >1 sync-wait. Move the
    TileContext tail-drain waits onto single-wait SP nops."""

    def _drain_and_barrier(self, tick_clock, wait_clock):
        drain_inst = self.nc.sync.drain()
        wait_clock.add_sem_waits(
            drain_inst.ins, ScopedClock({None: tick_clock.global_clock})
        )
        si = drain_inst.ins.sync_info
        if si is not None and len(si.on_wait) > 1:
            waits = list(si.on_wait)
            drain_inst.ins.sync_info = mybir.SyncInfo(
                on_wait=[], on_update=list(si.on_update)
            )
            for w in waits:
                nop = self.nc.sync.nop(nofuse=True)
                nop.ins.sync_info = mybir.SyncInfo(on_wait=[w], on_update=[])
        self.nc.all_engine_barrier()
        assert self.sems is not None
        popped = self.nc._tile_sem_poison_stack.pop()
        assert popped is self._sem_poison
        self.nc.clear_and_free_semaphores(list(self.sems.allocated().values()))
        self.nc.all_engine_barrier()

    TileContext._drain_and_barrier = _drain_and_barrier


def _install_ntff_hook():
    """antenv.axon_hooks is absent in this image; inject it and register the
    ctypes NTFF hook so trace=True / BASS_TRACE can profile."""
    try:
        import antenv

        if "antenv.axon_hooks" not in sys.modules:
            mod = types.ModuleType("antenv.axon_hooks")
            mod._hook = None

            def set_axon_ntff_profile_hook(h):
                mod._hook = h

            def get_axon_ntff_profile_hook():
                return mod._hook

            mod.set_axon_ntff_profile_hook = set_axon_ntff_profile_hook
            mod.get_axon_ntff_profile_hook = get_axon_ntff_profile_hook
            sys.modules["antenv.axon_hooks"] = mod
            antenv.axon_hooks = mod
        boot_dir = "/root/.axon_site/trn_agent_boot"
        so_path = "/opt/axon/libaxon_pjrt.so"
        if os.path.isdir(boot_dir) and os.path.exists(so_path):
            if boot_dir not in sys.path:
                sys.path.append(boot_dir)
            from trn_boot import _ntff_profile_via_ctypes

            hook = _ntff_profile_via_ctypes(so_path)
            if hook is not None:
                sys.modules["antenv.axon_hooks"].set_axon_ntff_profile_hook(hook)
    except Exception:
        pass


_patch_tile_drain()
_install_ntff_hook()


def _dram(nc, name, shape, dtype=F32, out=False):
    return nc.declare_dram_parameter(name, list(shape), dtype, isOutput=out)


def build_nc():
    nc = bacc_mod.Bacc("TRN2")

    xbT = _dram(nc, "xbT", [HID, S], BF16)            # x[b].T
    wd_kvT = _dram(nc, "wd_kvT", [HID, LAT], BF16)    # Wkv_down.T
    wd_qT = _dram(nc, "wd_qT", [HID, LAT], BF16)      # Wq_down.T
    wkr2T = _dram(nc, "wkr2T", [HID, 2 * R], BF16)    # [Wk_rope; rot].T
    wk_pT = _dram(nc, "wk_pT", [LAT, GH * C], BF16)   # 4-head k_c pack .T
    wv_pT = _dram(nc, "wv_pT", [LAT, GH * DH], BF16)  # 4-head v pack .T
    wqc_pT = _dram(nc, "wqc_pT", [LAT, GH * C], BF16)   # 4-head q_c pack /sqrt
    wqr_pT = _dram(nc, "wqr_pT", [LAT, GH * R], BF16)   # 4-head q_rope /sqrt
    wqrr_pT = _dram(nc, "wqrr_pT", [LAT, GH * R], BF16)  # rotated rope /sqrt
    woT = _dram(nc, "woT", [GH * DH, HID], BF16)      # Wo cols for our heads
    cos4_d = _dram(nc, "cos4", [128, S], BF16)        # cos.T tiled 4x
    sin4_d = _dram(nc, "sin4", [128, S], BF16)
    mask4_d = _dram(nc, "mask4", [128, NQC * 512], BF16)  # 0/1 diag masks
    out_d = _dram(nc, "out", [S, HID], out=True)      # partial (4-head) proj

    xbT_r = xbT[:, :].rearrange("(c p two) t -> c p two t", p=128, two=2)
    wd_kvT_r = wd_kvT[:, :].rearrange("(c p two) l -> c p two l", p=128, two=2)
    wd_qT_r = wd_qT[:, :].rearrange("(c p two) l -> c p two l", p=128, two=2)
    wkr2T_r = wkr2T[:, :].rearrange("(c p two) r -> c p two r", p=128, two=2)
    wk_pT_r = wk_pT[:, :].rearrange("(lc p) d -> lc p d", p=128)
    wv_pT_r = wv_pT[:, :].rearrange("(lc p) d -> lc p d", p=128)
    wqc_pT_r = wqc_pT[:, :].rearrange("(lc p) d -> lc p d", p=128)
    wqr_pT_r = wqr_pT[:, :].rearrange("(lc p) d -> lc p d", p=128)
    wqrr_pT_r = wqrr_pT[:, :].rearrange("(lc p) d -> lc p d", p=128)
    woT_r = woT[:, :].rearrange("(hc p) o -> hc p o", p=128)

    with TileContext(nc) as tc:
        with tc.tile_pool(name="perB", bufs=1) as perB, \
             tc.tile_pool(name="lat", bufs=2) as LATP, \
             tc.tile_pool(name="xs", bufs=1) as XS, \
             tc.tile_pool(name="ets", bufs=6) as ETS, \
             tc.tile_pool(name="acc", bufs=8) as ACC, \
             tc.tile_pool(name="rcp", bufs=2) as RCP, \
             tc.tile_pool(name="tmp", bufs=2) as TMP, \
             tc.tile_pool(name="ot", bufs=3) as OT, \
             tc.tile_pool(name="ps_g", bufs=3, space="PSUM") as PSG, \
             tc.tile_pool(name="ps_s", bufs=3, space="PSUM") as PSS, \
             tc.tile_pool(name="ps_c", bufs=2, space="PSUM") as PSC:

            # ---------- persistent SBUF ----------
            krT = perB.tile([32, S], BF16, tag="krT", name="krT")
            kT = perB.tile([128, GH, S], BF16, tag="kT", name="kT")
            vG = perB.tile([128, NKC, GH * DH], BF16, tag="vG", name="vG")
            qT = perB.tile([128, GH, S], BF16, tag="qT", name="qT")
            ctxT = perB.tile([128, GH, S], BF16, tag="ctxT", name="ctxT")
            cos4 = perB.tile([128, S], BF16, tag="cos4", name="cos4")
            sin4 = perB.tile([128, S], BF16, tag="sin4", name="sin4")
            mask4 = perB.tile([128, NQC * 512], BF16, tag="mask4", name="mask4")
            onesb = perB.tile([128, 128], BF16, tag="ones", name="ones")
            wk_sb = perB.tile([128, 4, GH * C], BF16, tag="wk", name="wk")
            wv_sb = perB.tile([128, 4, GH * DH], BF16, tag="wv", name="wv")
            wqc_sb = perB.tile([128, 4, GH * C], BF16, tag="wqc", name="wqc")
            wqr_sb = perB.tile([128, 4, GH * R], BF16, tag="wqr", name="wqr")
            wqrr_sb = perB.tile([128, 4, GH * R], BF16, tag="wqrr", name="wqrr")

            # down-proj weights: released after phase A(3), wo loaded after.
            # Per-hc tiles so the first matmuls wait only on their own slice;
            # kv weights first (the very first accumulation pass).
            WD = tc.alloc_tile_pool(name="wd", bufs=1, side="right")
            wdkv = [WD.tile([128, 2, LAT], BF16, tag=f"wdkv{hc}",
                            name=f"wdkv{hc}") for hc in range(8)]
            wdq = [WD.tile([128, 2, LAT], BF16, tag=f"wdq{hc}",
                           name=f"wdq{hc}") for hc in range(8)]
            wkr = [WD.tile([128, 2, 2 * R], BF16, tag=f"wkr{hc}",
                           name=f"wkr{hc}") for hc in range(8)]

            def load_wd():
                for hc in range(8):
                    nc.sync.dma_start(wdkv[hc][:], wd_kvT_r[hc])
                for hc in range(8):
                    nc.sync.dma_start(wkr[hc][:], wkr2T_r[hc])
                    nc.sync.dma_start(wdq[hc][:], wd_qT_r[hc])

            def load_x(tq):
                tsl = slice(tq * 512, (tq + 1) * 512)
                xt = [XS.tile([128, 2, 512], BF16, tag=f"xf{hc}",
                              name=f"xf{hc}") for hc in range(8)]
                for hc in range(8):
                    nc.sync.dma_start(xt[hc][:], xbT_r[hc][:, :, tsl])
                return xt

            WO = [None]  # box for the late wo pool
            wo_sb = [None]

            def load_cossin():
                nc.sync.dma_start(cos4[:], cos4_d[:, :])
                nc.sync.dma_start(sin4[:], sin4_d[:, :])

            def load_aux_weights():
                nc.sync.dma_start(mask4[:], mask4_d[:, :])
                nc.gpsimd.memset(onesb[:], 1.0)
                for lc in range(4):
                    nc.sync.dma_start(wk_sb[:, lc, :], wk_pT_r[lc])
                    nc.sync.dma_start(wv_sb[:, lc, :], wv_pT_r[lc])
                    nc.sync.dma_start(wqc_sb[:, lc, :], wqc_pT_r[lc])
                    nc.sync.dma_start(wqr_sb[:, lc, :], wqr_pT_r[lc])
                    nc.sync.dma_start(wqrr_sb[:, lc, :], wqrr_pT_r[lc])

            # ---------------- phase emitters ----------------
            def phA(tq, xt=None):
                """latents for token quarter tq: kv_lat, roped k_rope, q_lat.
                Returns the per-quarter latent tiles for phB(tq)."""
                tsl = slice(tq * 512, (tq + 1) * 512)
                if xt is None:
                    xt = load_x(tq)
                kv_t = LATP.tile([128, 4, 512], BF16, tag="kvlat",
                                 name="kvlat")
                q_t = LATP.tile([128, 4, 512], BF16, tag="qlat", name="qlat")

                # kv_lat: 4 lc passes, 2 rotating psum banks
                for lc in range(4):
                    ps = PSG.tile([128, 512], F32, tag="g", name=f"pkv{lc}")
                    for hc in range(8):
                        for two in range(2):
                            nc.tensor.matmul(
                                ps[:],
                                lhsT=wdkv[hc][:, two, lc * 128:(lc + 1) * 128],
                                rhs=xt[hc][:, two, :],
                                start=(hc == 0 and two == 0),
                                stop=(hc == 7 and two == 1),
                            )
                    nc.vector.tensor_copy(kv_t[:, lc, :], ps[:])
                # k_rope pass (64 rows: [rope; rot]); combine in place
                pkr = PSG.tile([64, 512], F32, tag="g", name="pkr")
                for hc in range(8):
                    for two in range(2):
                        nc.tensor.matmul(
                            pkr[:],
                            lhsT=wkr[hc][:, two, :],
                            rhs=xt[hc][:, two, :],
                            start=(hc == 0 and two == 0),
                            stop=(hc == 7 and two == 1),
                        )
                nc.vector.tensor_mul(pkr[0:32, :], pkr[0:32, :],
                                     cos4[0:32, tsl])
                tkr = TMP.tile([32, 512], F32, tag="tkr", name="tkr")
                nc.vector.tensor_mul(tkr[:], pkr[32:64, :], sin4[0:32, tsl])
                nc.vector.tensor_add(krT[:, tsl], pkr[0:32, :], tkr[:])
                # q_lat: 4 lc passes
                for lc in range(4):
                    ps = PSG.tile([128, 512], F32, tag="g", name=f"pq{lc}")
                    for hc in range(8):
                        for two in range(2):
                            nc.tensor.matmul(
                                ps[:],
                                lhsT=wdq[hc][:, two, lc * 128:(lc + 1) * 128],
                                rhs=xt[hc][:, two, :],
                                start=(hc == 0 and two == 0),
                                stop=(hc == 7 and two == 1),
                            )
                    nc.vector.tensor_copy(q_t[:, lc, :], ps[:])
                return kv_t, q_t

            def phB_gen(tq, kv_t, q_t):
                """per-head projections for quarter tq, as a generator of
                ~0.9us PE pieces (one psum group each)."""
                tsl = slice(tq * 512, (tq + 1) * 512)
                # k_c per head (96 content rows)
                for h in range(GH):
                    ps = PSG.tile([128, 512], F32, tag="g", name=f"pk{h}")
                    for lc in range(4):
                        nc.tensor.matmul(
                            ps[0:C, :],
                            lhsT=wk_sb[:, lc, h * C:(h + 1) * C],
                            rhs=kv_t[:, lc, :],
                            start=(lc == 0), stop=(lc == 3),
                        )
                    nc.vector.tensor_copy(kT[0:C, h, tsl], ps[0:C, :])
                    nc.sync.dma_start(kT[C:128, h, tsl], krT[:, tsl])
                    yield
                # v: 4 token sub-chunks of 128, out = [t, 4h*128]
                for t2 in range(4):
                    kc = tq * 4 + t2
                    ps = PSG.tile([128, 512], F32, tag="g", name=f"pv{t2}")
                    for lc in range(4):
                        nc.tensor.matmul(
                            ps[:],
                            lhsT=kv_t[:, lc, t2 * 128:(t2 + 1) * 128],
                            rhs=wv_sb[:, lc, :],
                            start=(lc == 0), stop=(lc == 3),
                        )
                    nc.vector.tensor_copy(vG[:, kc, :], ps[:])
                    yield
                # q_c per head
                for h in range(GH):
                    ps = PSG.tile([128, 512], F32, tag="g", name=f"pqc{h}")
                    for lc in range(4):
                        nc.tensor.matmul(
                            ps[0:C, :],
                            lhsT=wqc_sb[:, lc, h * C:(h + 1) * C],
                            rhs=q_t[:, lc, :],
                            start=(lc == 0), stop=(lc == 3),
                        )
                    nc.vector.tensor_copy(qT[0:C, h, tsl], ps[0:C, :])
                    yield
                # q_rope: stacked 4h x 32 rope + rot; combine, scatter
                psr = PSG.tile([128, 512], F32, tag="g", name="pqr")
                psrr = PSG.tile([128, 512], F32, tag="g", name="pqrr")
                for lc in range(4):
                    nc.tensor.matmul(
                        psr[:], lhsT=wqr_sb[:, lc, :],
                        rhs=q_t[:, lc, :],
                        start=(lc == 0), stop=(lc == 3),
                    )
                yield
                for lc in range(4):
                    nc.tensor.matmul(
                        psrr[:], lhsT=wqrr_sb[:, lc, :],
                        rhs=q_t[:, lc, :],
                        start=(lc == 0), stop=(lc == 3),
                    )
                t2b = TMP.tile([128, 512], F32, tag="t2b", name="t2b")
                t3 = TMP.tile([128, 512], BF16, tag="t3b", name="t3b")
                nc.vector.tensor_mul(psr[:], psr[:], cos4[:, tsl])
                nc.vector.tensor_mul(t2b[:], psrr[:], sin4[:, tsl])
                nc.vector.tensor_add(t3[:], psr[:], t2b[:])
                for h in range(GH):
                    nc.sync.dma_start(
                        qT[C:128, h, tsl], t3[32 * h:32 * h + 32, :]
                    )
                yield

            def phC_head(c, h, micro=None, spacing=1, ctr=None):
                """attention main for (chunk c, head h): scores+exp+ctx+tree.
                Returns state for phC_fin.  Diagonal key chunks last so their
                mask-multiply stays off the exp->ctx critical path.  `micro`
                is a generator of small exp-independent PE emissions, consumed
                every `spacing` ctx matmuls to absorb the ACT lag."""
                csl = slice(c * 512, (c + 1) * 512)
                nkc = 4 * (c + 1)
                kcs = list(range(4 * c)) + [4 * c + d for d in range(4)]
                ets = {}
                stack = []  # binary-counter tree: list of (level, tile)

                def emit_score(kc, i):
                    ps = PSS.tile([128, 512], F32, tag="s", name=f"ps{i % 2}")
                    nc.tensor.matmul(
                        ps[:],
                        lhsT=kT[:, h, kc * 128:(kc + 1) * 128],
                        rhs=qT[:, h, csl],
                        start=True, stop=True,
                    )
                    et = ETS.tile([128, 512], BF16, tag="e", name=f"et{i % 6}")
                    nc.scalar.activation(
                        et[:], ps[:], mybir.ActivationFunctionType.Exp
                    )
                    off = kc - 4 * c
                    if off >= 0:
                        nc.vector.tensor_mul(
                            et[:], et[:], mask4[:, off * 512:(off + 1) * 512]
                        )
                    ets[kc] = et

                pctx = PSC.tile([128, 512], F32, tag="c", name="pctx")

                def emit_ctx(kc, i):
                    nc.tensor.matmul(
                        pctx[:],
                        lhsT=vG[:, kc, h * DH:(h + 1) * DH],
                        rhs=ets[kc][:],
                        start=(i == 0), stop=(i == nkc - 1),
                    )
                    # fold into the tree-sum (bf16); level-0 pair adds go to
                    # the otherwise-idle GpSimd, upper levels to DVE
                    carry = ets[kc]
                    lvl = 0
                    while stack and stack[-1][0] == lvl:
                        _, other = stack.pop()
                        dst = ACC.tile([128, 512], BF16, tag="a",
                                       name=f"acc{i % 5}")
                        nc.vector.tensor_add(dst[:], other[:], carry[:])
                        carry = dst
                        lvl += 1
                    stack.append((lvl, carry))

                def tick():
                    if micro is not None and ctr is not None:
                        ctr[0] += 1
                        if ctr[0] % spacing == 0:
                            next(micro, None)

                LAG = 3
                for i, kc in enumerate(kcs):
                    emit_score(kc, i)
                    if i >= LAG:
                        emit_ctx(kcs[i - LAG], i - LAG)
                        tick()
                for i in range(max(0, nkc - LAG), nkc):
                    emit_ctx(kcs[i], i)
                    tick()
                # fold remaining tree levels
                while len(stack) > 1:
                    l1, a = stack.pop()
                    l2, b = stack.pop()
                    dst = ACC.tile([128, 512], BF16, tag="a", name="accf")
                    nc.vector.tensor_add(dst[:], a[:], b[:])
                    stack.append((max(l1, l2) + 1, dst))
                return pctx, stack[0][1]

            def phC_fin(c, h, pctx, tsum):
                """row-sum via all-ones matmul, reciprocal, ctx normalize."""
                csl = slice(c * 512, (c + 1) * 512)
                prs = PSC.tile([128, 512], F32, tag="c", name="prs")
                nc.tensor.matmul(
                    prs[:], lhsT=onesb[:], rhs=tsum[:], start=True, stop=True
                )
                rc = RCP.tile([128, 512], F32, tag="rc", name="rc")
                nc.vector.reciprocal_approx_fast(out=rc[:], in_=prs[:])
                nc.vector.tensor_mul(ctxT[:, h, csl], pctx[:], rc[:])

            def phC(c):
                for h in range(GH):
                    pctx, tsum = phC_head(c, h)
                    phC_fin(c, h, pctx, tsum)

            def load_wo():
                if WO[0] is None:
                    WO[0] = tc.alloc_tile_pool(name="wo", bufs=1, side="right")
                    wo_sb[0] = WO[0].tile([128, 4, HID], BF16, tag="wo",
                                          name="wo")
                    for hc in range(4):
                        nc.sync.dma_start(wo_sb[0][:, hc, :], woT_r[hc])

            def phD_oc(qb, oc, on_act=False):
                """one [128q, 512o] psum group of the partial out-proj.
                phD runs only after phA(3), so it owns the PSG banks; in the
                exp-free tail the evac copy goes on the idle ACT engine."""
                ps = PSG.tile([128, 512], F32, tag="g", name=f"po{oc % 3}")
                for h in range(GH):
                    nc.tensor.matmul(
                        ps[:],
                        lhsT=ctxT[:, h, qb * 128:(qb + 1) * 128],
                        rhs=wo_sb[0][:, h, oc * 512:(oc + 1) * 512],
                        start=(h == 0), stop=(h == 3),
                    )
                ot = OT.tile([128, 512], F32, tag="ot", name="ot")
                if on_act:
                    nc.scalar.copy(ot[:], ps[:])
                else:
                    nc.vector.tensor_copy(ot[:], ps[:])
                nc.sync.dma_start(
                    out_d[qb * 128:(qb + 1) * 128, oc * 512:(oc + 1) * 512],
                    ot[:],
                )

            def phD_qb(qb, on_act=False):
                """partial out-projection for one 128-row query block."""
                for oc in range(4):
                    phD_oc(qb, oc, on_act)

            def phD_gen(qbs):
                for qb in qbs:
                    for oc in range(4):
                        phD_oc(qb, oc)
                        yield

            # ---------------- master schedule ----------------
            xt0 = load_x(0)
            load_wd()
            load_cossin()
            # warm-up matmuls on a memset tile: PE starts at ~+2us (no DMA
            # dependency), so HAM is un-throttled before the real work
            wmt = perB.tile([128, 512], BF16, tag="wmt", name="wmt")
            nc.gpsimd.memset(wmt[:], 0.0)
            for i in range(26):
                pw = PSG.tile([128, 512], F32, tag="g", name=f"warm{i % 2}")
                nc.tensor.matmul(pw[:], lhsT=wmt[:, 0:128], rhs=wmt[:],
                                 start=True, stop=True)
            from itertools import chain as _chain

            lat0 = phA(0, xt0)
            load_aux_weights()
            for _ in phB_gen(0, *lat0):
                pass
            lat1 = phA(1)

            def run_chunk(c, micro, spacing):
                ctr = [0]
                for h in range(GH):
                    st = phC_head(c, h, micro, spacing, ctr)
                    phC_fin(c, h, *st)
                for _ in micro:   # drain leftover pieces
                    pass

            # micro-fillers keep the PE fed while ACT does the exps
            run_chunk(0, phB_gen(1, *lat1), 1)
            lat2 = phA(2)
            run_chunk(1, phB_gen(2, *lat2), 2)
            lat3 = phA(3)
            WD.release()
            load_wo()
            run_chunk(2, _chain(phB_gen(3, *lat3), phD_gen([0])), 2)
            run_chunk(3, phD_gen(range(1, 9)), 2)
            for qb in range(9, 16):
                phD_qb(qb, on_act=(qb % 2 == 1))
            if WO[0] is not None:
                WO[0].release()

    nc.compile()
    return nc


def _rot_rows(w):
    # rows of w are the rope dim; rot(w) @ lat == rotate_half(w @ lat)
    hR = w.shape[0] // 2
    return np.concatenate([-w[hR:], w[:hR]], axis=0)


def _prep_inputs(inputs):
    x = np.asarray(inputs["x"], np.float32)
    Wq_down = np.asarray(inputs["Wq_down"], np.float32)
    Wq_up = np.asarray(inputs["Wq_up"], np.float32)
    Wq_rope = np.asarray(inputs["Wq_rope"], np.float32)
    Wkv_down = np.asarray(inputs["Wkv_down"], np.float32)
    Wk_up = np.asarray(inputs["Wk_up"], np.float32)
    Wk_rope = np.asarray(inputs["Wk_rope"], np.float32)
    Wv_up = np.asarray(inputs["Wv_up"], np.float32)
    Wo = np.asarray(inputs["Wo"], np.float32)

    s = np.float32(1.0 / np.sqrt(DH))

    wd_kvT = np.ascontiguousarray(Wkv_down.T).astype(BF16NP)
    wd_qT = np.ascontiguousarray(Wq_down.T).astype(BF16NP)
    wkr2 = np.concatenate([Wk_rope, _rot_rows(Wk_rope)], axis=0)  # [64, HID]
    wkr2T = np.ascontiguousarray(wkr2.T).astype(BF16NP)

    inv_freq = (1.0 / (10000.0 ** (np.arange(0, R, 2, dtype=np.float32) / R)))
    t = np.arange(S, dtype=np.float32)
    freqs = t[:, None] * inv_freq[None, :]
    emb = np.concatenate([freqs, freqs], axis=-1)          # [S, R]
    cos4 = np.tile(np.cos(emb).T, (4, 1)).astype(BF16NP)   # [128, S]
    sin4 = np.tile(np.sin(emb).T, (4, 1)).astype(BF16NP)

    kar = np.arange(128)[:, None]
    qar = np.arange(512)[None, :]
    mask4 = np.empty((128, NQC * 512), np.float32)
    for off in range(4):
        mask4[:, off * 512:(off + 1) * 512] = (
            (128 * off + kar) <= qar
        ).astype(np.float32)
    mask4 = mask4.astype(BF16NP)

    per_g = []
    for g in range(4):
        hsl = slice(g * GH, (g + 1) * GH)
        wk_p = np.concatenate(
            [Wk_up[h * C:(h + 1) * C] for h in range(g * GH, (g + 1) * GH)],
            axis=0)                                         # [384, LAT]
        wv_p = np.concatenate(
            [Wv_up[h * DH:(h + 1) * DH] for h in range(g * GH, (g + 1) * GH)],
            axis=0)                                         # [512, LAT]
        wqc_p = np.concatenate(
            [Wq_up[h * C:(h + 1) * C] for h in range(g * GH, (g + 1) * GH)],
            axis=0) * s
        wqr_p = np.concatenate(
            [Wq_rope[h * R:(h + 1) * R] for h in range(g * GH, (g + 1) * GH)],
            axis=0) * s
        wqrr_p = np.concatenate(
            [_rot_rows(Wq_rope[h * R:(h + 1) * R])
             for h in range(g * GH, (g + 1) * GH)], axis=0) * s
        wo_g = Wo[:, g * GH * DH:(g + 1) * GH * DH]         # [HID, 512]
        per_g.append({
            "wk_pT": np.ascontiguousarray(wk_p.T).astype(BF16NP),
            "wv_pT": np.ascontiguousarray(wv_p.T).astype(BF16NP),
            "wqc_pT": np.ascontiguousarray(wqc_p.T).astype(BF16NP),
            "wqr_pT": np.ascontiguousarray(wqr_p.T).astype(BF16NP),
            "wqrr_pT": np.ascontiguousarray(wqrr_p.T).astype(BF16NP),
            "woT": np.ascontiguousarray(wo_g.T).astype(BF16NP),
        })

    in_maps = []
    for cid in range(8):
        b, g = divmod(cid, 4)
        m = {
            "xbT": np.ascontiguousarray(x[b].T).astype(BF16NP),
            "wd_kvT": wd_kvT, "wd_qT": wd_qT, "wkr2T": wkr2T,
            "cos4": cos4, "sin4": sin4, "mask4": mask4,
        }
        m.update(per_g[g])
        in_maps.append(m)
    return in_maps


_NC_CACHE = None


def kernel(**inputs):
    global _NC_CACHE
    if _NC_CACHE is None:
        _NC_CACHE = build_nc()
    nc = _NC_CACHE
    in_maps = _prep_inputs(inputs)
    res = run_bass_kernel_spmd(nc, in_maps, list(range(8)))
    bo = np.asarray(inputs["bo"], np.float32)
    out = np.empty((B, S, HID), np.float32)
    for b in range(B):
        acc = res.results[4 * b]["out"].astype(np.float32)
        for g in range(1, 4):
            acc = acc + res.results[4 * b + g]["out"]
        out[b] = acc + bo
    return out
